# revision 18
# baseline (speedup 1.0000x reference)
"""ClassAttention kernel for 8x TRN2 NeuronCores — fp8 DoubleRow rewrite.

Reference computation (per batch element):
    qkv = x @ qkv_w.T + qkv_b                      # [N, 3C]
    q, k, v = split(qkv)                           # heads H=12, D=64
    s = softmax((q_cls . k) / sqrt(D))             # class-token query only
    cls = (s @ v) @ proj_w.T + proj_b              # [1, C]
    out = concat([cls, x[1:]])                     # rows 1..N pass through

Only the class token row changes, so the device computes just the [B, C]
cls output (shipped transposed as clsT [C, B]); rows 1..N pass through on
the host.  Data-parallel over batch: 8 batches per core, no collectives.

Algebraic structure (inherited from the bf16 baseline):
  - k-projection folds into x-space:  s[b,h,n] = sum_c Wt[c,bh] x[b,n,c]
    with Wt = wk.T @ blockdiag(q) computed once on device; no k vector is
    materialized.  k-bias cancels in softmax; q-bias folds into Wt via a
    host-precomputed wtqb.
  - v-projection commutes with the attention average: the kernel averages
    x (ZT = x.T @ p) and projects through wv once; v-bias folds into the
    proj bias on the host.
  - softmax skips the max-shift (scores are O(1)); the 1/sum scaling is
    applied per (b,h) column during the ZT psum evacuation.

What is new vs the baseline (82.2us -> ~35us modeled):
  - fp8(e4m3) data path: x (both layouts), wv, wp, Wt, p=exp(s), ZT, oT
    are fp8; the score-weight path (wq, wk2, q, Qblk, Wt accumulation)
    stays bf16 because it dominates the error budget.  DoubleRow fp8
    matmuls (2 K-tiles per instruction, 0.5 cycles/row) carry all the
    heavy contractions.
  - every stage computes the TRANSPOSED output with a small moving free
    dim (qT, sT, ZT, oT, clsT), so there are ZERO data transposes and
    psum evacuations are few and wide ([128, .] copies, not [12, .]).
  - 21 large DMAs instead of 67 (HWDGE issue cost ~630ns each gated the
    baseline); x2 is read as [128, 5, 768] per batch from a 63-row-padded
    flat buffer so each batch is one descriptor-dense transfer.

Per-core dataflow (b = 0..8 batches, c in 6 chunks of 128):
  qT[o, b]        36 bf16 matmuls      (needs xcls, wq)
  Qblk[o, (b h)]  12 blockdiag copies  (DVE, psum -> bf16)
  Wt[c, (b h)]    36 bf16 matmuls + 6 adds (+wtqb, cast fp8)
  sT[n, (b h)]    120 DR matmuls       (needs all xT)
  pT = exp(sT-1)  2 Act ops, fp8       (bias cancels in the 1/sum)
  sums[1, (b h)]  3 ones-matmuls; rden = 1/sums (f32)
  rdenB[o, (b h)] 2 outer-product matmuls + copy
  ZT[c, (g j h)]  144 DR matmuls       (needs x2_b), x rden -> fp8
  oT[o', b]       72 DR matmuls        (diag blocks direct, needs wv)
  clsT[j, b]      36 DR matmuls + pbT add -> f32, DMA out per group
"""

import functools

import numpy as np
import ml_dtypes

import concourse.bass as bass
import concourse.tile as tile
from concourse import bacc, mybir
from concourse import bass_utils

BF16 = mybir.dt.bfloat16
F8 = mybir.dt.float8e4
F32 = mybir.dt.float32
NPBF16 = ml_dtypes.bfloat16
NPF8 = ml_dtypes.float8_e4m3
DR = mybir.MatmulPerfMode.DoubleRow

B, N, C = 64, 577, 768
H, D = 12, 64
NCORES = 8
BPC = B // NCORES          # 8 batches per core
CT = C // 128              # 6 chunks of the feature dim
NT = 5                     # token tiles of 128 (last holds 65)
NTAIL = N - 4 * 128        # 65
SCALE = D ** -0.5          # folded into wq on the host
X2PAD = 5 * 128 - N        # 63 rows of row padding after the last batch


def build_module():
    nc = bacc.Bacc("TRN2", target_bir_lowering=False, debug=False)

    xT_d = nc.dram_tensor("xT", [C, BPC, N], F8, kind="ExternalInput")
    x2_d = nc.dram_tensor("x2", [BPC * N + X2PAD, C], F8, kind="ExternalInput")
    wq_d = nc.dram_tensor("wq", [C, C], BF16, kind="ExternalInput")    # [c, o]
    wk2_d = nc.dram_tensor("wk2", [C, C], BF16, kind="ExternalInput")  # [o, c]
    wv_d = nc.dram_tensor("wv", [C, C], F8, kind="ExternalInput")      # [c, o]
    wp_d = nc.dram_tensor("wp", [C, C], F8, kind="ExternalInput")      # [c, o]
    xcls_d = nc.dram_tensor("xcls", [C, BPC], BF16, kind="ExternalInput")
    wtqb_d = nc.dram_tensor("wtqb", [C, BPC * H], BF16, kind="ExternalInput")
    pbT_d = nc.dram_tensor("pbT", [C, BPC], F32, kind="ExternalInput")
    clsT_d = nc.dram_tensor("clsT", [C, BPC], F32, kind="ExternalOutput")

    AF = mybir.ActivationFunctionType

    with tile.TileContext(nc) as tc:
        with (
            tc.tile_pool(name="sb", bufs=1) as sb,
            tc.tile_pool(name="psA", bufs=2, space="PSUM") as psA,
            tc.tile_pool(name="psW", bufs=2, space="PSUM") as psW,
            tc.tile_pool(name="psS", bufs=1, space="PSUM") as psS,
            tc.tile_pool(name="psR", bufs=1, space="PSUM") as psR,
            tc.tile_pool(name="psZ", bufs=2, space="PSUM") as psZ,
        ):
            # ---- DMAs, in consumption order (one channel, serialized) ----
            xcls = sb.tile([128, CT, BPC], BF16, tag="xcls")
            nc.sync.dma_start(
                xcls[:], xcls_d.ap().rearrange("(a p) b -> p a b", p=128))
            wq = sb.tile([128, CT, C], BF16, tag="wq")
            nc.sync.dma_start(
                wq[:], wq_d.ap().rearrange("(a p) o -> p a o", p=128))
            wk2 = sb.tile([128, CT, C], BF16, tag="wk2")
            nc.sync.dma_start(
                wk2[:], wk2_d.ap().rearrange("(a p) o -> p a o", p=128))
            wtqb = sb.tile([128, CT, BPC * H], BF16, tag="wtqb")
            nc.sync.dma_start(
                wtqb[:], wtqb_d.ap().rearrange("(a p) o -> p a o", p=128))
            # x in c-major layout, one DMA per batch; rows padded to 640 so
            # DoubleRow k-tile-pair slices have a 64-multiple stride (walrus
            # ISA requirement on Ldweights)
            xTs = []
            for b in range(BPC):
                xt = sb.tile([128, CT, 640], F8, tag=f"xT{b}")
                nc.sync.dma_start(
                    xt[:, :, 0:N],
                    xT_d.ap()[:, b, :].rearrange("(a p) t -> p a t", p=128))
                xTs.append(xt)
            wv = sb.tile([128, CT, C], F8, tag="wv")
            nc.sync.dma_start(
                wv[:], wv_d.ap().rearrange("(a p) o -> p a o", p=128))
            wp = sb.tile([128, CT, C], F8, tag="wp")
            nc.sync.dma_start(
                wp[:], wp_d.ap().rearrange("(a p) o -> p a o", p=128))
            pbT = sb.tile([128, CT, BPC], F32, tag="pbT")
            nc.sync.dma_start(
                pbT[:], pbT_d.ap().rearrange("(a p) b -> p a b", p=128))
            # x in token-major layout, one overlapping [640, C] read per
            # batch (rows past token 577 belong to the next batch / the host
            # pad and are masked by exact-K tail matmuls)
            x2s = []
            for b in range(BPC):
                x2 = sb.tile([128, NT, C], F8, tag=f"x2{b}")
                nc.sync.dma_start(
                    x2[:],
                    x2_d.ap()[b * N:b * N + NT * 128, :]
                    .rearrange("(a p) c -> p a c", p=128))
                x2s.append(x2)

            # ---- small constants ----
            ones8 = sb.tile([128, 2, 64], F8, tag="ones8")
            nc.vector.memset(ones8[:], 1.0)
            negone = sb.tile([128, 1], F32, tag="negone")
            nc.vector.memset(negone[:], -1.0)
            onesf = sb.tile([1, 128], F32, tag="onesf")
            nc.vector.memset(onesf[:], 1.0)
            Qblk = sb.tile([128, CT, BPC * H], BF16, tag="Qblk")
            nc.vector.memset(Qblk[:], 0.0)

            # fp8 operand tiles are padded so every DoubleRow k-pair slice
            # has a 64-multiple stride
            Wt = sb.tile([128, CT, 128], F8, tag="Wt")
            pT = sb.tile([128, NT, BPC, 16], F8, tag="pT")
            rden = sb.tile([1, BPC * H], F32, tag="rden")
            rdenB = sb.tile([128, 2, 1, 48], F32, tag="rdenB")
            ZT = sb.tile([128, CT, 2, 4, 16], F8, tag="ZT")
            oT = sb.tile([128, CT, 64], F8, tag="oT")
            clsT_sb = sb.tile([128, CT, BPC], F32, tag="clsT_sb")

            # ---- qT[o, b]: 36 bf16 matmuls, out free dim 8 ----
            pq = psA.tile([128, CT, BPC], F32, tag="A")
            for oc in range(CT):
                for ck in range(CT):
                    nc.tensor.matmul(
                        pq[:, oc, :],
                        wq[:, ck, 128 * oc:128 * (oc + 1)],
                        xcls[:, ck, :],
                        start=(ck == 0), stop=(ck == CT - 1))

            # ---- Qblk[o, (b h)]: blockdiag scatter of qT (bf16) ----
            QblkV = Qblk[:].rearrange("p a (b h) -> p a b h", h=H)
            for oc in range(CT):
                for j in range(2):
                    h = 2 * oc + j
                    nc.vector.tensor_copy(
                        QblkV[64 * j:64 * (j + 1), oc, :, h],
                        pq[64 * j:64 * (j + 1), oc, :])

            # ---- Wt[c, (b h)] = wk2.T @ Qblk + wtqb, cast fp8 ----
            for cj in range(CT):
                pw = psW.tile([128, BPC * H], F32, tag="W")
                for ok in range(CT):
                    nc.tensor.matmul(
                        pw[:], wk2[:, ok, 128 * cj:128 * (cj + 1)],
                        Qblk[:, ok, :],
                        start=(ok == 0), stop=(ok == CT - 1))
                nc.vector.tensor_add(Wt[:, cj, 0:BPC * H], pw[:], wtqb[:, cj, :])

            # ---- sT[n, (b h)] per batch: 15 DR matmuls over c ----
            ps_s = psS.tile([128, NT, BPC, H], F32, tag="S")
            for b in range(BPC):
                for nt in range(NT):
                    w = 128 if nt < NT - 1 else NTAIL
                    off = 128 * nt
                    for t in range(3):
                        nc.tensor.matmul(
                            ps_s[:w, nt, b, :],
                            xTs[b][:, 2 * t:2 * t + 2, off:off + w],
                            Wt[:, 2 * t:2 * t + 2, H * b:H * (b + 1)],
                            start=(t == 0), stop=(t == 2), perf_mode=DR)

            # ---- pT = exp(sT - 1), fp8 (the -1 cancels in 1/sum and
            #      keeps e below the fp8e4 max) ----
            nc.scalar.activation(
                pT[:, 0:4, :, 0:H], ps_s[:, 0:4, :, :], AF.Exp,
                bias=negone[:], scale=1.0)
            nc.scalar.activation(
                pT[:NTAIL, 4, :, 0:H], ps_s[:NTAIL, 4, :, :], AF.Exp,
                bias=negone[:NTAIL, :], scale=1.0)

            # ---- sums over n via ones-matmuls; rden = 1/sums ----
            pr = psR.tile([128, 192], F32, tag="R")
            for nt in range(NT):
                w = 128 if nt < NT - 1 else NTAIL
                nc.tensor.matmul(
                    pr[0:1, 0:96], ones8[:w, 0, 0:1],
                    pT[:w, nt, :, 0:H],
                    start=(nt == 0), stop=(nt == NT - 1))
            nc.vector.reciprocal(rden[:], pr[0:1, 0:96])

            # ---- rdenB[o, (j h)] per group: broadcast rden down 128
            #      partitions with an outer-product matmul ----
            for g in range(2):
                nc.tensor.matmul(
                    pr[:, 96 + 48 * g:96 + 48 * (g + 1)],
                    onesf[:], rden[0:1, 48 * g:48 * (g + 1)],
                    start=True, stop=True)
            nc.vector.tensor_copy(
                rdenB[:].rearrange("p g u c -> p (g u c)"), pr[:, 96:192])

            # ---- ZT[c, (j h)] per (group, batch): 18 DR matmuls +
            #      normalize-and-cast evacuation ----
            for g in range(2):
                pz = psZ.tile([128, CT, 4, H], F32, tag="Z")
                for jj in range(4):
                    b = 4 * g + jj
                    x2 = x2s[b]
                    for ci in range(CT):
                        for t in range(2):
                            nc.tensor.matmul(
                                pz[:, ci, jj, :],
                                x2[:, 2 * t:2 * t + 2,
                                   128 * ci:128 * (ci + 1)],
                                pT[:, 2 * t:2 * t + 2, b, 0:H],
                                start=(t == 0), stop=False, perf_mode=DR)
                        nc.tensor.matmul(
                            pz[:, ci, jj, :],
                            x2[:NTAIL, 4, 128 * ci:128 * (ci + 1)],
                            pT[:NTAIL, 4, b, 0:H],
                            start=False, stop=True)
                nc.vector.tensor_mul(
                    ZT[:, :, g, :, 0:H],
                    pz[:],
                    rdenB[:, g, :, :].rearrange("p u (j h) -> p u j h", h=H)
                    .to_broadcast([128, CT, 4, H]))

                # ---- oT[o', b] for this group: head-diagonal blocks of
                #      wv.T @ ZT, computed directly (out free dim 4) ----
                if g == 0:
                    po = psA.tile([128, CT, BPC], F32, tag="A")
                # non-DR: DoubleRow + dst partition 64 fails the walrus ISA
                # check (s3d3_mm_valid_dst_partition); cost is per-out-column
                # anyway so plain fp8 matmuls are the same speed here
                for ci in range(CT):
                    for hh in range(2):
                        h = 2 * ci + hh
                        base = 128 * ci + 64 * hh
                        for t in range(CT):
                            nc.tensor.matmul(
                                po[64 * hh:64 * (hh + 1), ci, 4 * g:4 * (g + 1)],
                                wv[:, t, base:base + 64],
                                ZT[:, t, g, :, h],
                                start=(t == 0), stop=(t == CT - 1),
                                tile_position=(0, 64 * hh))
                nc.vector.tensor_copy(
                    oT[:, :, 4 * g:4 * (g + 1)],
                    po[:, :, 4 * g:4 * (g + 1)])

                # ---- clsT[j, b] = wp.T @ oT + pbT, f32 out ----
                if g == 0:
                    pc = psA.tile([128, CT, BPC], F32, tag="A")
                for jc in range(CT):
                    for t in range(3):
                        nc.tensor.matmul(
                            pc[:, jc, 4 * g:4 * (g + 1)],
                            wp[:, 2 * t:2 * t + 2, 128 * jc:128 * (jc + 1)],
                            oT[:, 2 * t:2 * t + 2, 4 * g:4 * (g + 1)],
                            start=(t == 0), stop=(t == 2), perf_mode=DR)
                nc.vector.tensor_add(
                    clsT_sb[:, :, 4 * g:4 * (g + 1)],
                    pc[:, :, 4 * g:4 * (g + 1)],
                    pbT[:, :, 4 * g:4 * (g + 1)])
                nc.sync.dma_start(
                    clsT_d.ap()[:, 4 * g:4 * (g + 1)]
                    .rearrange("(a p) b -> p a b", p=128),
                    clsT_sb[:, :, 4 * g:4 * (g + 1)])

    nc.compile()
    return nc


@functools.lru_cache(maxsize=1)
def _module():
    return build_module()


def make_in_maps(x, qkv_w, qkv_b, proj_w, proj_b):
    x = np.asarray(x, dtype=np.float32)
    qkv_w = np.asarray(qkv_w, dtype=np.float32)
    qkv_b = np.asarray(qkv_b, dtype=np.float32)
    proj_w = np.asarray(proj_w, dtype=np.float32)
    proj_b = np.asarray(proj_b, dtype=np.float32)

    wq = np.ascontiguousarray(qkv_w[:C].T * SCALE).astype(NPBF16)   # [c, o]
    wk2 = np.ascontiguousarray(qkv_w[C:2 * C]).astype(NPBF16)       # [o, c]
    wv = np.ascontiguousarray(qkv_w[2 * C:].T).astype(NPF8)         # [c, o]
    wp = np.ascontiguousarray(proj_w.T).astype(NPF8)                # [c, o]
    # q-bias folds into Wt: wtqb[c, h] = wk_block_h[:, c] . qb_block_h
    qbs = qkv_b[:C] * SCALE
    wtqb1 = np.stack(
        [qkv_w[C + 64 * h:C + 64 * (h + 1)].T @ qbs[64 * h:64 * (h + 1)]
         for h in range(H)], axis=1)                                # [C, H]
    wtqb = np.tile(wtqb1, (1, BPC)).astype(NPBF16)                  # [C, 96]
    # v bias contributes exactly (vb @ proj_w.T) to cls; fold into proj bias
    pb_eff = proj_b + qkv_b[2 * C:] @ proj_w.T

    in_maps = []
    for i in range(NCORES):
        xs = x[i * BPC:(i + 1) * BPC]                               # [8, N, C]
        x2 = np.zeros((BPC * N + X2PAD, C), dtype=NPF8)
        x2[:BPC * N] = xs.reshape(BPC * N, C).astype(NPF8)
        xT = np.ascontiguousarray(xs.transpose(2, 0, 1)).astype(NPF8)
        xcls = np.ascontiguousarray(xs[:, 0, :].T).astype(NPBF16)   # [C, 8]
        pbT = np.tile(pb_eff[:, None], (1, BPC)).astype(np.float32)
        in_maps.append({
            "xT": xT, "x2": x2, "wq": wq, "wk2": wk2, "wv": wv, "wp": wp,
            "xcls": xcls, "wtqb": wtqb, "pbT": pbT,
        })
    return in_maps


def kernel(x, qkv_w, qkv_b, proj_w, proj_b):
    nc = _module()
    in_maps = make_in_maps(x, qkv_w, qkv_b, proj_w, proj_b)
    res = bass_utils.run_bass_kernel_spmd(
        nc, in_maps, core_ids=list(range(NCORES)))
    out = np.array(np.asarray(x), dtype=np.float32, copy=True)
    for i in range(NCORES):
        out[i * BPC:(i + 1) * BPC, 0, :] = res.results[i]["clsT"].T
    return out


# revision 26
# speedup vs baseline: 1.0419x; 1.0419x over previous
"""ClassAttention kernel for 8x TRN2 NeuronCores — fp8 DoubleRow rewrite.

Reference computation (per batch element):
    qkv = x @ qkv_w.T + qkv_b                      # [N, 3C]
    q, k, v = split(qkv)                           # heads H=12, D=64
    s = softmax((q_cls . k) / sqrt(D))             # class-token query only
    cls = (s @ v) @ proj_w.T + proj_b              # [1, C]
    out = concat([cls, x[1:]])                     # rows 1..N pass through

Only the class token row changes, so the device computes just the [B, C]
cls output (shipped transposed as clsT [C, B]); rows 1..N pass through on
the host.  Data-parallel over batch: 8 batches per core, no collectives.

Algebraic structure (inherited from the bf16 baseline):
  - k-projection folds into x-space:  s[b,h,n] = sum_c Wt[c,bh] x[b,n,c]
    with Wt = wk.T @ blockdiag(q) computed once on device; no k vector is
    materialized.  k-bias cancels in softmax; q-bias folds into Wt via a
    host-precomputed wtqb.
  - v-projection commutes with the attention average: the kernel averages
    x (ZT = x.T @ p) and projects through wv once; v-bias folds into the
    proj bias on the host.
  - softmax skips the max-shift (scores are O(1)); the 1/sum scaling is
    applied per (b,h) column during the ZT psum evacuation.

What is new vs the baseline (82.2us -> ~35us modeled):
  - fp8(e4m3) data path: x (both layouts), wv, wp, Wt, p=exp(s), ZT, oT
    are fp8; the score-weight path (wq, wk2, q, Qblk, Wt accumulation)
    stays bf16 because it dominates the error budget.  DoubleRow fp8
    matmuls (2 K-tiles per instruction, 0.5 cycles/row) carry all the
    heavy contractions.
  - every stage computes the TRANSPOSED output with a small moving free
    dim (qT, sT, ZT, oT, clsT), so there are ZERO data transposes and
    psum evacuations are few and wide ([128, .] copies, not [12, .]).
  - 21 large DMAs instead of 67 (HWDGE issue cost ~630ns each gated the
    baseline); x2 is read as [128, 5, 768] per batch from a 63-row-padded
    flat buffer so each batch is one descriptor-dense transfer.

Per-core dataflow (b = 0..8 batches, c in 6 chunks of 128):
  qT[o, b]        36 bf16 matmuls      (needs xcls, wq)
  Qblk[o, (b h)]  12 blockdiag copies  (DVE, psum -> bf16)
  Wt[c, (b h)]    36 bf16 matmuls + 6 adds (+wtqb, cast fp8)
  sT[n, (b h)]    120 DR matmuls       (needs all xT)
  pT = exp(sT-1)  2 Act ops, fp8       (bias cancels in the 1/sum)
  sums[1, (b h)]  3 ones-matmuls; rden = 1/sums (f32)
  rdenB[o, (b h)] 2 outer-product matmuls + copy
  ZT[c, (g j h)]  144 DR matmuls       (needs x2_b), x rden -> fp8
  oT[o', b]       72 DR matmuls        (diag blocks direct, needs wv)
  clsT[j, b]      36 DR matmuls + pbT add -> f32, DMA out per group
"""

import functools

import numpy as np
import ml_dtypes

import concourse.bass as bass
import concourse.tile as tile
from concourse import bacc, mybir
from concourse import bass_utils

BF16 = mybir.dt.bfloat16
F8 = mybir.dt.float8e4
F32 = mybir.dt.float32
NPBF16 = ml_dtypes.bfloat16
NPF8 = ml_dtypes.float8_e4m3
DR = mybir.MatmulPerfMode.DoubleRow

B, N, C = 64, 577, 768
H, D = 12, 64
NCORES = 8
BPC = B // NCORES          # 8 batches per core
CT = C // 128              # 6 chunks of the feature dim
NT = 5                     # token tiles of 128 (last holds 65)
NTAIL = N - 4 * 128        # 65
SCALE = D ** -0.5          # folded into wq on the host
X2PAD = 5 * 128 - N        # 63 rows of row padding after the last batch


def build_module():
    nc = bacc.Bacc("TRN2", target_bir_lowering=False, debug=False)

    xT_d = nc.dram_tensor("xT", [C, BPC, N], F8, kind="ExternalInput")
    x2_d = nc.dram_tensor("x2", [BPC * N, C], F8, kind="ExternalInput")
    wq_d = nc.dram_tensor("wq", [C, C], BF16, kind="ExternalInput")    # [c, o]
    wk2_d = nc.dram_tensor("wk2", [C, C], BF16, kind="ExternalInput")  # [o, c]
    wv_d = nc.dram_tensor("wv", [C, C], F8, kind="ExternalInput")      # [c, o]
    wp_d = nc.dram_tensor("wp", [C, C], F8, kind="ExternalInput")      # [c, o]
    xcls_d = nc.dram_tensor("xcls", [C, BPC], BF16, kind="ExternalInput")
    wtqb_d = nc.dram_tensor("wtqb", [C, BPC * H], F8, kind="ExternalInput")
    pbT_d = nc.dram_tensor("pbT", [C, BPC], F32, kind="ExternalInput")
    clsT_d = nc.dram_tensor("clsT", [C, BPC], F32, kind="ExternalOutput")

    AF = mybir.ActivationFunctionType

    with tile.TileContext(nc) as tc:
        with (
            tc.tile_pool(name="sb", bufs=1) as sb,
            tc.tile_pool(name="psA", bufs=2, space="PSUM") as psA,
            tc.tile_pool(name="psW", bufs=1, space="PSUM") as psW,
            tc.tile_pool(name="psS", bufs=1, space="PSUM") as psS,
            tc.tile_pool(name="psR", bufs=1, space="PSUM") as psR,
            tc.tile_pool(name="psZ", bufs=3, space="PSUM") as psZ,
        ):
            # ---- DMAs, in consumption order (one channel, serialized) ----
            wq = sb.tile([128, CT, C], BF16, tag="wq")
            nc.sync.dma_start(
                wq[:], wq_d.ap().rearrange("(a p) o -> p a o", p=128))
            wk2 = sb.tile([128, CT, C], BF16, tag="wk2")
            nc.sync.dma_start(
                wk2[:], wk2_d.ap().rearrange("(a p) o -> p a o", p=128))
            xcls = sb.tile([128, CT, BPC], BF16, tag="xcls")
            nc.sync.dma_start(
                xcls[:], xcls_d.ap().rearrange("(a p) b -> p a b", p=128))
            wtqb = sb.tile([128, CT, BPC * H], F8, tag="wtqb")
            nc.sync.dma_start(
                wtqb[:], wtqb_d.ap().rearrange("(a p) o -> p a o", p=128))
            # x in c-major layout, one DMA per batch; rows padded to 640 so
            # DoubleRow k-tile-pair slices have a 64-multiple stride (walrus
            # ISA requirement on Ldweights)
            xTs = []
            for b in range(BPC):
                xt = sb.tile([128, CT, 640], F8, tag=f"xT{b}")
                nc.sync.dma_start(
                    xt[:, :, 0:N],
                    xT_d.ap()[:, b, :].rearrange("(a p) t -> p a t", p=128))
                xTs.append(xt)
            wv = sb.tile([128, CT, C], F8, tag="wv")
            nc.sync.dma_start(
                wv[:], wv_d.ap().rearrange("(a p) o -> p a o", p=128))
            wp = sb.tile([128, CT, C], F8, tag="wp")
            nc.sync.dma_start(
                wp[:], wp_d.ap().rearrange("(a p) o -> p a o", p=128))
            pbT = sb.tile([128, CT, BPC], F32, tag="pbT")
            nc.sync.dma_start(
                pbT[:], pbT_d.ap().rearrange("(a p) b -> p a b", p=128))
            # x in token-major layout, two exact-size DMAs per batch (the
            # 512-row body, then the 65-row tail) so the last batch's Z
            # matmuls mostly run before its tail lands
            x2s = []
            for b in range(BPC):
                x2 = sb.tile([128, NT, C], F8, tag=f"x2{b}")
                nc.sync.dma_start(
                    x2[:, 0:4, :],
                    x2_d.ap()[b * N:b * N + 512, :]
                    .rearrange("(a p) c -> p a c", p=128))
                nc.sync.dma_start(
                    x2[0:NTAIL, 4, :],
                    x2_d.ap()[b * N + 512:b * N + N, :])
                x2s.append(x2)

            # ---- small constants ----
            ones8 = sb.tile([128, 2, 64], F8, tag="ones8")
            nc.vector.memset(ones8[:], 1.0)
            negone = sb.tile([128, 1], F32, tag="negone")
            nc.vector.memset(negone[:], -1.0)
            onesf = sb.tile([1, 128], F32, tag="onesf")
            nc.vector.memset(onesf[:], 1.0)
            Qblk = sb.tile([128, CT, BPC * H], BF16, tag="Qblk")
            nc.vector.memset(Qblk[:], 0.0)

            # fp8 operand tiles are padded so every DoubleRow k-pair slice
            # has a 64-multiple stride
            Wt = sb.tile([128, CT, 128], F8, tag="Wt")
            pT = sb.tile([128, NT, BPC, 16], F8, tag="pT")
            rden = sb.tile([1, BPC * H], F32, tag="rden")
            rdenB = sb.tile([128, BPC, H], F32, tag="rdenB")
            ZT = sb.tile([128, CT, BPC, 16], F8, tag="ZT")
            oT = sb.tile([128, CT, 64], F8, tag="oT")
            clsT_sb = sb.tile([128, CT, BPC], F32, tag="clsT_sb")

            # ---- qT[o, b]: 36 bf16 matmuls, out free dim 8 ----
            pq = psA.tile([128, CT, BPC], F32, tag="A")
            for oc in range(CT):
                for ck in range(CT):
                    nc.tensor.matmul(
                        pq[:, oc, :],
                        wq[:, ck, 128 * oc:128 * (oc + 1)],
                        xcls[:, ck, :],
                        start=(ck == 0), stop=(ck == CT - 1))

            # ---- Qblk[o, (b h)]: blockdiag scatter of qT (bf16) ----
            QblkV = Qblk[:].rearrange("p a (b h) -> p a b h", h=H)
            for oc in range(CT):
                for j in range(2):
                    h = 2 * oc + j
                    nc.vector.tensor_copy(
                        QblkV[64 * j:64 * (j + 1), oc, :, h],
                        pq[64 * j:64 * (j + 1), oc, :])

            # ---- Wt[c, (b h)] = wk2.T @ Qblk + wtqb, cast fp8 ----
            for cj in range(CT):
                pw = psW.tile([128, BPC * H], F32, tag="W")
                for ok in range(CT):
                    nc.tensor.matmul(
                        pw[:], wk2[:, ok, 128 * cj:128 * (cj + 1)],
                        Qblk[:, ok, :],
                        start=(ok == 0), stop=(ok == CT - 1))
                nc.vector.tensor_add(Wt[:, cj, 0:BPC * H], pw[:], wtqb[:, cj, :])

            # ---- sT[n, (b h)] per batch: 15 DR matmuls over c ----
            ps_s = psS.tile([128, NT, BPC, H], F32, tag="S")
            for b in range(BPC):
                for nt in range(NT):
                    w = 128 if nt < NT - 1 else NTAIL
                    off = 128 * nt
                    for t in range(3):
                        nc.tensor.matmul(
                            ps_s[:w, nt, b, :],
                            xTs[b][:, 2 * t:2 * t + 2, off:off + w],
                            Wt[:, 2 * t:2 * t + 2, H * b:H * (b + 1)],
                            start=(t == 0), stop=(t == 2), perf_mode=DR)

            # ---- pT = exp(sT - 1), fp8 (the -1 cancels in 1/sum and
            #      keeps e below the fp8e4 max) ----
            nc.scalar.activation(
                pT[:, 0:4, :, 0:H], ps_s[:, 0:4, :, :], AF.Exp,
                bias=negone[:], scale=1.0)
            nc.scalar.activation(
                pT[:NTAIL, 4, :, 0:H], ps_s[:NTAIL, 4, :, :], AF.Exp,
                bias=negone[:NTAIL, :], scale=1.0)

            # ---- sums over n via ones-matmuls; rden = 1/sums ----
            pr = psR.tile([128, 192], F32, tag="R")
            for nt in range(NT):
                w = 128 if nt < NT - 1 else NTAIL
                nc.tensor.matmul(
                    pr[0:1, 0:96], ones8[:w, 0, 0:1],
                    pT[:w, nt, :, 0:H],
                    start=(nt == 0), stop=(nt == NT - 1))
            nc.vector.reciprocal(rden[:], pr[0:1, 0:96])

            # ---- rdenB[o, (b h)]: broadcast rden down 128 partitions with
            #      an outer-product matmul ----
            nc.tensor.matmul(
                pr[:, 96:192], onesf[:], rden[:], start=True, stop=True)
            nc.vector.tensor_copy(
                rdenB[:].rearrange("p b h -> p (b h)"), pr[:, 96:192])

            # ---- ZT[c, b-col] per batch: 18 DR matmuls + normalize-and-
            #      cast evacuation (runs as each x2 batch lands) ----
            po = psA.tile([128, CT, BPC], F32, tag="A")
            for b in range(BPC):
                pz = psZ.tile([128, CT, H], F32, tag="Z")
                x2 = x2s[b]
                for ci in range(CT):
                    for t in range(2):
                        nc.tensor.matmul(
                            pz[:, ci, :],
                            x2[:, 2 * t:2 * t + 2, 128 * ci:128 * (ci + 1)],
                            pT[:, 2 * t:2 * t + 2, b, 0:H],
                            start=(t == 0), stop=False, perf_mode=DR)
                    nc.tensor.matmul(
                        pz[:, ci, :],
                        x2[:NTAIL, 4, 128 * ci:128 * (ci + 1)],
                        pT[:NTAIL, 4, b, 0:H],
                        start=False, stop=True)
                nc.vector.tensor_mul(
                    ZT[:, :, b, 0:H], pz[:],
                    rdenB[:, b:b + 1, :].to_broadcast([128, CT, H]))

            # ---- per tail group (4/3/1 batches): oT, clsT, output DMA.
            #      The last group is a single batch so the post-stream
            #      dependency chain is as short as possible. ----
            pc = psA.tile([128, CT, BPC], F32, tag="A")
            for js, jn in ((0, 4), (4, 3), (7, 1)):
                # oT[o', b]: head-diagonal blocks of wv.T @ ZT, directly.
                # non-DR: DoubleRow + dst partition 64 fails the walrus ISA
                # check (s3d3_mm_valid_dst_partition); cost is per-out-column
                # anyway so plain fp8 matmuls are the same speed here
                for ci in range(CT):
                    for hh in range(2):
                        h = 2 * ci + hh
                        base = 128 * ci + 64 * hh
                        for t in range(CT):
                            nc.tensor.matmul(
                                po[64 * hh:64 * (hh + 1), ci, js:js + jn],
                                wv[:, t, base:base + 64],
                                ZT[:, t, js:js + jn, h],
                                start=(t == 0), stop=(t == CT - 1),
                                tile_position=(0, 64 * hh))
                nc.vector.tensor_copy(
                    oT[:, :, js:js + jn], po[:, :, js:js + jn])

                # clsT[j, b] = wp.T @ oT + pbT, f32 out
                for jc in range(CT):
                    for t in range(3):
                        nc.tensor.matmul(
                            pc[:, jc, js:js + jn],
                            wp[:, 2 * t:2 * t + 2, 128 * jc:128 * (jc + 1)],
                            oT[:, 2 * t:2 * t + 2, js:js + jn],
                            start=(t == 0), stop=(t == 2), perf_mode=DR)
                nc.vector.tensor_add(
                    clsT_sb[:, :, js:js + jn],
                    pc[:, :, js:js + jn],
                    pbT[:, :, js:js + jn])
                nc.sync.dma_start(
                    clsT_d.ap()[:, js:js + jn]
                    .rearrange("(a p) b -> p a b", p=128),
                    clsT_sb[:, :, js:js + jn])

    nc.compile()
    return nc


@functools.lru_cache(maxsize=1)
def _module():
    return build_module()


def make_in_maps(x, qkv_w, qkv_b, proj_w, proj_b):
    x = np.asarray(x, dtype=np.float32)
    qkv_w = np.asarray(qkv_w, dtype=np.float32)
    qkv_b = np.asarray(qkv_b, dtype=np.float32)
    proj_w = np.asarray(proj_w, dtype=np.float32)
    proj_b = np.asarray(proj_b, dtype=np.float32)

    wq = np.ascontiguousarray(qkv_w[:C].T * SCALE).astype(NPBF16)   # [c, o]
    wk2 = np.ascontiguousarray(qkv_w[C:2 * C]).astype(NPBF16)       # [o, c]
    wv = np.ascontiguousarray(qkv_w[2 * C:].T).astype(NPF8)         # [c, o]
    wp = np.ascontiguousarray(proj_w.T).astype(NPF8)                # [c, o]
    # q-bias folds into Wt: wtqb[c, h] = wk_block_h[:, c] . qb_block_h
    qbs = qkv_b[:C] * SCALE
    wtqb1 = np.stack(
        [qkv_w[C + 64 * h:C + 64 * (h + 1)].T @ qbs[64 * h:64 * (h + 1)]
         for h in range(H)], axis=1)                                # [C, H]
    wtqb = np.tile(wtqb1, (1, BPC)).astype(NPF8)                    # [C, 96]
    # v bias contributes exactly (vb @ proj_w.T) to cls; fold into proj bias
    pb_eff = proj_b + qkv_b[2 * C:] @ proj_w.T

    in_maps = []
    for i in range(NCORES):
        xs = x[i * BPC:(i + 1) * BPC]                               # [8, N, C]
        x2 = xs.reshape(BPC * N, C).astype(NPF8)
        xT = np.ascontiguousarray(xs.transpose(2, 0, 1)).astype(NPF8)
        xcls = np.ascontiguousarray(xs[:, 0, :].T).astype(NPBF16)   # [C, 8]
        pbT = np.tile(pb_eff[:, None], (1, BPC)).astype(np.float32)
        in_maps.append({
            "xT": xT, "x2": x2, "wq": wq, "wk2": wk2, "wv": wv, "wp": wp,
            "xcls": xcls, "wtqb": wtqb, "pbT": pbT,
        })
    return in_maps


def kernel(x, qkv_w, qkv_b, proj_w, proj_b):
    nc = _module()
    in_maps = make_in_maps(x, qkv_w, qkv_b, proj_w, proj_b)
    res = bass_utils.run_bass_kernel_spmd(
        nc, in_maps, core_ids=list(range(NCORES)))
    out = np.array(np.asarray(x), dtype=np.float32, copy=True)
    for i in range(NCORES):
        out[i * BPC:(i + 1) * BPC, 0, :] = res.results[i]["clsT"].T
    return out


# revision 28
# speedup vs baseline: 1.0960x; 1.0520x over previous
"""ClassAttention kernel for 8x TRN2 NeuronCores — fp8 DoubleRow rewrite.

Reference computation (per batch element):
    qkv = x @ qkv_w.T + qkv_b                      # [N, 3C]
    q, k, v = split(qkv)                           # heads H=12, D=64
    s = softmax((q_cls . k) / sqrt(D))             # class-token query only
    cls = (s @ v) @ proj_w.T + proj_b              # [1, C]
    out = concat([cls, x[1:]])                     # rows 1..N pass through

Only the class token row changes, so the device computes just the [B, C]
cls output (shipped transposed as clsT [C, B]); rows 1..N pass through on
the host.  Data-parallel over batch: 8 batches per core, no collectives.

Algebraic structure (inherited from the bf16 baseline):
  - k-projection folds into x-space:  s[b,h,n] = sum_c Wt[c,bh] x[b,n,c]
    with Wt = wk.T @ blockdiag(q) computed once on device; no k vector is
    materialized.  k-bias cancels in softmax; q-bias folds into Wt via a
    host-precomputed wtqb.
  - v-projection commutes with the attention average: the kernel averages
    x (ZT = x.T @ p) and projects through wv once; v-bias folds into the
    proj bias on the host.
  - softmax skips the max-shift (scores are O(1)); the 1/sum scaling is
    applied per (b,h) column during the ZT psum evacuation.

What is new vs the baseline (82.2us -> ~35us modeled):
  - fp8(e4m3) data path: x (both layouts), wv, wp, Wt, p=exp(s), ZT, oT
    are fp8; the score-weight path (wq, wk2, q, Qblk, Wt accumulation)
    stays bf16 because it dominates the error budget.  DoubleRow fp8
    matmuls (2 K-tiles per instruction, 0.5 cycles/row) carry all the
    heavy contractions.
  - every stage computes the TRANSPOSED output with a small moving free
    dim (qT, sT, ZT, oT, clsT), so there are ZERO data transposes and
    psum evacuations are few and wide ([128, .] copies, not [12, .]).
  - 21 large DMAs instead of 67 (HWDGE issue cost ~630ns each gated the
    baseline); x2 is read as [128, 5, 768] per batch from a 63-row-padded
    flat buffer so each batch is one descriptor-dense transfer.

Per-core dataflow (b = 0..8 batches, c in 6 chunks of 128):
  qT[o, b]        36 bf16 matmuls      (needs xcls, wq)
  Qblk[o, (b h)]  12 blockdiag copies  (DVE, psum -> bf16)
  Wt[c, (b h)]    36 bf16 matmuls + 6 adds (+wtqb, cast fp8)
  sT[n, (b h)]    120 DR matmuls       (needs all xT)
  pT = exp(sT-1)  2 Act ops, fp8       (bias cancels in the 1/sum)
  sums[1, (b h)]  3 ones-matmuls; rden = 1/sums (f32)
  rdenB[o, (b h)] 2 outer-product matmuls + copy
  ZT[c, (g j h)]  144 DR matmuls       (needs x2_b), x rden -> fp8
  oT[o', b]       72 DR matmuls        (diag blocks direct, needs wv)
  clsT[j, b]      36 DR matmuls + pbT add -> f32, DMA out per group
"""

import functools

import numpy as np
import ml_dtypes

import concourse.bass as bass
import concourse.tile as tile
from concourse import bacc, mybir
from concourse import bass_utils

BF16 = mybir.dt.bfloat16
F8 = mybir.dt.float8e4
F32 = mybir.dt.float32
NPBF16 = ml_dtypes.bfloat16
NPF8 = ml_dtypes.float8_e4m3
DR = mybir.MatmulPerfMode.DoubleRow

B, N, C = 64, 577, 768
H, D = 12, 64
NCORES = 8
BPC = B // NCORES          # 8 batches per core
CT = C // 128              # 6 chunks of the feature dim
NT = 5                     # token tiles of 128 (last holds 65)
NTAIL = N - 4 * 128        # 65
SCALE = D ** -0.5          # folded into wq on the host
X2PAD = 5 * 128 - N        # 63 rows of row padding after the last batch


def build_module():
    nc = bacc.Bacc("TRN2", target_bir_lowering=False, debug=False)

    xT_d = nc.dram_tensor("xT", [C, BPC, N], F8, kind="ExternalInput")
    x2_d = nc.dram_tensor("x2", [BPC * N, C], F8, kind="ExternalInput")
    wq_d = nc.dram_tensor("wq", [C, C], BF16, kind="ExternalInput")    # [c, o]
    wk2_d = nc.dram_tensor("wk2", [C, C], BF16, kind="ExternalInput")  # [o, c]
    wv_d = nc.dram_tensor("wv", [C, C], F8, kind="ExternalInput")      # [c, o]
    wp_d = nc.dram_tensor("wp", [C, C], F8, kind="ExternalInput")      # [c, o]
    xcls_d = nc.dram_tensor("xcls", [C, BPC], BF16, kind="ExternalInput")
    wtqb_d = nc.dram_tensor("wtqb", [C, BPC * H], F8, kind="ExternalInput")
    pbT_d = nc.dram_tensor("pbT", [C, BPC], F32, kind="ExternalInput")
    clsT_d = nc.dram_tensor("clsT", [C, BPC], F32, kind="ExternalOutput")

    AF = mybir.ActivationFunctionType

    with tile.TileContext(nc) as tc:
        with (
            tc.tile_pool(name="sb", bufs=1) as sb,
            tc.tile_pool(name="psA", bufs=2, space="PSUM") as psA,
            tc.tile_pool(name="psW", bufs=1, space="PSUM") as psW,
            tc.tile_pool(name="psS", bufs=1, space="PSUM") as psS,
            tc.tile_pool(name="psR", bufs=1, space="PSUM") as psR,
            tc.tile_pool(name="psZ", bufs=3, space="PSUM") as psZ,
        ):
            # ---- DMAs, in consumption order (one channel, serialized) ----
            wq = sb.tile([128, CT, C], BF16, tag="wq")
            nc.sync.dma_start(
                wq[:], wq_d.ap().rearrange("(a p) o -> p a o", p=128))
            wk2 = sb.tile([128, CT, C], BF16, tag="wk2")
            nc.sync.dma_start(
                wk2[:], wk2_d.ap().rearrange("(a p) o -> p a o", p=128))
            xcls = sb.tile([128, CT, BPC], BF16, tag="xcls")
            nc.sync.dma_start(
                xcls[:], xcls_d.ap().rearrange("(a p) b -> p a b", p=128))
            wtqb = sb.tile([128, CT, BPC * H], F8, tag="wtqb")
            nc.sync.dma_start(
                wtqb[:], wtqb_d.ap().rearrange("(a p) o -> p a o", p=128))
            # x in c-major layout, one DMA per batch; rows padded to 640 so
            # DoubleRow k-tile-pair slices have a 64-multiple stride (walrus
            # ISA requirement on Ldweights)
            pbT = sb.tile([128, CT, BPC], F32, tag="pbT")
            nc.sync.dma_start(
                pbT[:], pbT_d.ap().rearrange("(a p) b -> p a b", p=128))
            wv = sb.tile([128, CT, C], F8, tag="wv")
            nc.sync.dma_start(
                wv[:], wv_d.ap().rearrange("(a p) o -> p a o", p=128))
            xTs = []
            for b in range(BPC):
                xt = sb.tile([128, CT, 640], F8, tag=f"xT{b}")
                nc.sync.dma_start(
                    xt[:, :, 0:N],
                    xT_d.ap()[:, b, :].rearrange("(a p) t -> p a t", p=128))
                xTs.append(xt)
            # x in token-major layout, two exact-size DMAs per batch (the
            # 512-row body, then the 65-row tail) so the last batch's Z
            # matmuls mostly run before its tail lands
            x2s = []
            for b in range(BPC):
                x2 = sb.tile([128, NT, C], F8, tag=f"x2{b}")
                nc.sync.dma_start(
                    x2[:, 0:4, :],
                    x2_d.ap()[b * N:b * N + 512, :]
                    .rearrange("(a p) c -> p a c", p=128))
                nc.sync.dma_start(
                    x2[0:NTAIL, 4, :],
                    x2_d.ap()[b * N + 512:b * N + N, :])
                x2s.append(x2)
            # wp is the LAST input: everything up to oT overlaps the input
            # stream, so the only post-stream work is proj -> add -> out DMA
            wp = sb.tile([128, CT, C], F8, tag="wp")
            nc.sync.dma_start(
                wp[:], wp_d.ap().rearrange("(a p) o -> p a o", p=128))

            # ---- small constants ----
            ones8 = sb.tile([128, 2, 64], F8, tag="ones8")
            nc.vector.memset(ones8[:], 1.0)
            negone = sb.tile([128, 1], F32, tag="negone")
            nc.vector.memset(negone[:], -1.0)
            onesf = sb.tile([1, 128], F32, tag="onesf")
            nc.vector.memset(onesf[:], 1.0)
            Qblk = sb.tile([128, CT, BPC * H], BF16, tag="Qblk")
            nc.vector.memset(Qblk[:], 0.0)

            # fp8 operand tiles are padded so every DoubleRow k-pair slice
            # has a 64-multiple stride
            Wt = sb.tile([128, CT, 128], F8, tag="Wt")
            pT = sb.tile([128, NT, BPC, 16], F8, tag="pT")
            rden = sb.tile([1, BPC * H], F32, tag="rden")
            rdenB = sb.tile([128, BPC, H], F32, tag="rdenB")
            ZT = sb.tile([128, CT, BPC, 16], F8, tag="ZT")
            oT = sb.tile([128, CT, 64], F8, tag="oT")
            clsT_sb = sb.tile([128, CT, BPC], F32, tag="clsT_sb")

            # ---- qT[o, b]: 36 bf16 matmuls, out free dim 8 ----
            pq = psA.tile([128, CT, BPC], F32, tag="A")
            for oc in range(CT):
                for ck in range(CT):
                    nc.tensor.matmul(
                        pq[:, oc, :],
                        wq[:, ck, 128 * oc:128 * (oc + 1)],
                        xcls[:, ck, :],
                        start=(ck == 0), stop=(ck == CT - 1))

            # ---- Qblk[o, (b h)]: blockdiag scatter of qT (bf16) ----
            QblkV = Qblk[:].rearrange("p a (b h) -> p a b h", h=H)
            for oc in range(CT):
                for j in range(2):
                    h = 2 * oc + j
                    nc.vector.tensor_copy(
                        QblkV[64 * j:64 * (j + 1), oc, :, h],
                        pq[64 * j:64 * (j + 1), oc, :])

            # ---- Wt[c, (b h)] = wk2.T @ Qblk + wtqb, cast fp8 ----
            for cj in range(CT):
                pw = psW.tile([128, BPC * H], F32, tag="W")
                for ok in range(CT):
                    nc.tensor.matmul(
                        pw[:], wk2[:, ok, 128 * cj:128 * (cj + 1)],
                        Qblk[:, ok, :],
                        start=(ok == 0), stop=(ok == CT - 1))
                nc.vector.tensor_add(Wt[:, cj, 0:BPC * H], pw[:], wtqb[:, cj, :])

            # ---- sT[n, (b h)] per batch: 15 DR matmuls over c ----
            ps_s = psS.tile([128, NT, BPC, H], F32, tag="S")
            for b in range(BPC):
                for nt in range(NT):
                    w = 128 if nt < NT - 1 else NTAIL
                    off = 128 * nt
                    for t in range(3):
                        nc.tensor.matmul(
                            ps_s[:w, nt, b, :],
                            xTs[b][:, 2 * t:2 * t + 2, off:off + w],
                            Wt[:, 2 * t:2 * t + 2, H * b:H * (b + 1)],
                            start=(t == 0), stop=(t == 2), perf_mode=DR)

            # ---- pT = exp(sT - 1), fp8 (the -1 cancels in 1/sum and
            #      keeps e below the fp8e4 max) ----
            nc.scalar.activation(
                pT[:, 0:4, :, 0:H], ps_s[:, 0:4, :, :], AF.Exp,
                bias=negone[:], scale=1.0)
            nc.scalar.activation(
                pT[:NTAIL, 4, :, 0:H], ps_s[:NTAIL, 4, :, :], AF.Exp,
                bias=negone[:NTAIL, :], scale=1.0)

            # ---- sums over n via ones-matmuls; rden = 1/sums ----
            pr = psR.tile([128, 192], F32, tag="R")
            for nt in range(NT):
                w = 128 if nt < NT - 1 else NTAIL
                nc.tensor.matmul(
                    pr[0:1, 0:96], ones8[:w, 0, 0:1],
                    pT[:w, nt, :, 0:H],
                    start=(nt == 0), stop=(nt == NT - 1))
            nc.vector.reciprocal(rden[:], pr[0:1, 0:96])

            # ---- rdenB[o, (b h)]: broadcast rden down 128 partitions with
            #      an outer-product matmul ----
            nc.tensor.matmul(
                pr[:, 96:192], onesf[:], rden[:], start=True, stop=True)
            nc.vector.tensor_copy(
                rdenB[:].rearrange("p b h -> p (b h)"), pr[:, 96:192])

            # ---- ZT[c, b-col] per batch: 18 DR matmuls + normalize-and-
            #      cast evacuation (runs as each x2 batch lands) ----
            po = psA.tile([128, CT, BPC], F32, tag="A")
            for b in range(BPC):
                pz = psZ.tile([128, CT, H], F32, tag="Z")
                x2 = x2s[b]
                for ci in range(CT):
                    for t in range(2):
                        nc.tensor.matmul(
                            pz[:, ci, :],
                            x2[:, 2 * t:2 * t + 2, 128 * ci:128 * (ci + 1)],
                            pT[:, 2 * t:2 * t + 2, b, 0:H],
                            start=(t == 0), stop=False, perf_mode=DR)
                    nc.tensor.matmul(
                        pz[:, ci, :],
                        x2[:NTAIL, 4, 128 * ci:128 * (ci + 1)],
                        pT[:NTAIL, 4, b, 0:H],
                        start=False, stop=True)
                nc.vector.tensor_mul(
                    ZT[:, :, b, 0:H], pz[:],
                    rdenB[:, b:b + 1, :].to_broadcast([128, CT, H]))

            # ---- oT per group (4/3/1 batches) so it tracks x2 arrivals.
            # non-DR: DoubleRow + dst partition 64 fails the walrus ISA
            # check (s3d3_mm_valid_dst_partition); cost is per-out-column
            # anyway so plain fp8 matmuls are the same speed here ----
            for js, jn in ((0, 4), (4, 3), (7, 1)):
                for ci in range(CT):
                    for hh in range(2):
                        h = 2 * ci + hh
                        base = 128 * ci + 64 * hh
                        for t in range(CT):
                            nc.tensor.matmul(
                                po[64 * hh:64 * (hh + 1), ci, js:js + jn],
                                wv[:, t, base:base + 64],
                                ZT[:, t, js:js + jn, h],
                                start=(t == 0), stop=(t == CT - 1),
                                tile_position=(0, 64 * hh))
                nc.vector.tensor_copy(
                    oT[:, :, js:js + jn], po[:, :, js:js + jn])

            # ---- clsT[j, b] = wp.T @ oT + pbT: the only work that waits
            #      for wp (the last DMA); one add, one output DMA ----
            pc = psA.tile([128, CT, BPC], F32, tag="A")
            for jc in range(CT):
                for t in range(3):
                    nc.tensor.matmul(
                        pc[:, jc, :],
                        wp[:, 2 * t:2 * t + 2, 128 * jc:128 * (jc + 1)],
                        oT[:, 2 * t:2 * t + 2, 0:BPC],
                        start=(t == 0), stop=(t == 2), perf_mode=DR)
            nc.vector.tensor_add(clsT_sb[:], pc[:], pbT[:])
            nc.sync.dma_start(
                clsT_d.ap().rearrange("(a p) b -> p a b", p=128),
                clsT_sb[:])

    nc.compile()
    return nc


@functools.lru_cache(maxsize=1)
def _module():
    return build_module()


def make_in_maps(x, qkv_w, qkv_b, proj_w, proj_b):
    x = np.asarray(x, dtype=np.float32)
    qkv_w = np.asarray(qkv_w, dtype=np.float32)
    qkv_b = np.asarray(qkv_b, dtype=np.float32)
    proj_w = np.asarray(proj_w, dtype=np.float32)
    proj_b = np.asarray(proj_b, dtype=np.float32)

    wq = np.ascontiguousarray(qkv_w[:C].T * SCALE).astype(NPBF16)   # [c, o]
    wk2 = np.ascontiguousarray(qkv_w[C:2 * C]).astype(NPBF16)       # [o, c]
    wv = np.ascontiguousarray(qkv_w[2 * C:].T).astype(NPF8)         # [c, o]
    wp = np.ascontiguousarray(proj_w.T).astype(NPF8)                # [c, o]
    # q-bias folds into Wt: wtqb[c, h] = wk_block_h[:, c] . qb_block_h
    qbs = qkv_b[:C] * SCALE
    wtqb1 = np.stack(
        [qkv_w[C + 64 * h:C + 64 * (h + 1)].T @ qbs[64 * h:64 * (h + 1)]
         for h in range(H)], axis=1)                                # [C, H]
    wtqb = np.tile(wtqb1, (1, BPC)).astype(NPF8)                    # [C, 96]
    # v bias contributes exactly (vb @ proj_w.T) to cls; fold into proj bias
    pb_eff = proj_b + qkv_b[2 * C:] @ proj_w.T

    in_maps = []
    for i in range(NCORES):
        xs = x[i * BPC:(i + 1) * BPC]                               # [8, N, C]
        x2 = xs.reshape(BPC * N, C).astype(NPF8)
        xT = np.ascontiguousarray(xs.transpose(2, 0, 1)).astype(NPF8)
        xcls = np.ascontiguousarray(xs[:, 0, :].T).astype(NPBF16)   # [C, 8]
        pbT = np.tile(pb_eff[:, None], (1, BPC)).astype(np.float32)
        in_maps.append({
            "xT": xT, "x2": x2, "wq": wq, "wk2": wk2, "wv": wv, "wp": wp,
            "xcls": xcls, "wtqb": wtqb, "pbT": pbT,
        })
    return in_maps


def kernel(x, qkv_w, qkv_b, proj_w, proj_b):
    nc = _module()
    in_maps = make_in_maps(x, qkv_w, qkv_b, proj_w, proj_b)
    res = bass_utils.run_bass_kernel_spmd(
        nc, in_maps, core_ids=list(range(NCORES)))
    out = np.array(np.asarray(x), dtype=np.float32, copy=True)
    for i in range(NCORES):
        out[i * BPC:(i + 1) * BPC, 0, :] = res.results[i]["clsT"].T
    return out


# revision 30
# speedup vs baseline: 1.1466x; 1.0461x over previous
"""ClassAttention kernel for 8x TRN2 NeuronCores — fp8 DoubleRow rewrite.

Reference computation (per batch element):
    qkv = x @ qkv_w.T + qkv_b                      # [N, 3C]
    q, k, v = split(qkv)                           # heads H=12, D=64
    s = softmax((q_cls . k) / sqrt(D))             # class-token query only
    cls = (s @ v) @ proj_w.T + proj_b              # [1, C]
    out = concat([cls, x[1:]])                     # rows 1..N pass through

Only the class token row changes, so the device computes just the [B, C]
cls output (shipped transposed as clsT [C, B]); rows 1..N pass through on
the host.  Data-parallel over batch: 8 batches per core, no collectives.

Algebraic structure (inherited from the bf16 baseline):
  - k-projection folds into x-space:  s[b,h,n] = sum_c Wt[c,bh] x[b,n,c]
    with Wt = wk.T @ blockdiag(q) computed once on device; no k vector is
    materialized.  k-bias cancels in softmax; q-bias folds into Wt via a
    host-precomputed wtqb.
  - v-projection commutes with the attention average: the kernel averages
    x (ZT = x.T @ p) and projects through wv once; v-bias folds into the
    proj bias on the host.
  - softmax skips the max-shift (scores are O(1)); the 1/sum scaling is
    applied per (b,h) column during the ZT psum evacuation.

What is new vs the baseline (82.2us -> ~35us modeled):
  - fp8(e4m3) data path: x (both layouts), wv, wp, Wt, p=exp(s), ZT, oT
    are fp8; the score-weight path (wq, wk2, q, Qblk, Wt accumulation)
    stays bf16 because it dominates the error budget.  DoubleRow fp8
    matmuls (2 K-tiles per instruction, 0.5 cycles/row) carry all the
    heavy contractions.
  - every stage computes the TRANSPOSED output with a small moving free
    dim (qT, sT, ZT, oT, clsT), so there are ZERO data transposes and
    psum evacuations are few and wide ([128, .] copies, not [12, .]).
  - 21 large DMAs instead of 67 (HWDGE issue cost ~630ns each gated the
    baseline); x2 is read as [128, 5, 768] per batch from a 63-row-padded
    flat buffer so each batch is one descriptor-dense transfer.

Per-core dataflow (b = 0..8 batches, c in 6 chunks of 128):
  qT[o, b]        36 bf16 matmuls      (needs xcls, wq)
  Qblk[o, (b h)]  12 blockdiag copies  (DVE, psum -> bf16)
  Wt[c, (b h)]    36 bf16 matmuls + 6 adds (+wtqb, cast fp8)
  sT[n, (b h)]    120 DR matmuls       (needs all xT)
  pT = exp(sT-1)  2 Act ops, fp8       (bias cancels in the 1/sum)
  sums[1, (b h)]  3 ones-matmuls; rden = 1/sums (f32)
  rdenB[o, (b h)] 2 outer-product matmuls + copy
  ZT[c, (g j h)]  144 DR matmuls       (needs x2_b), x rden -> fp8
  oT[o', b]       72 DR matmuls        (diag blocks direct, needs wv)
  clsT[j, b]      36 DR matmuls + pbT add -> f32, DMA out per group
"""

import functools

import numpy as np
import ml_dtypes

import concourse.bass as bass
import concourse.tile as tile
from concourse import bacc, mybir
from concourse import bass_utils

BF16 = mybir.dt.bfloat16
F8 = mybir.dt.float8e4
F32 = mybir.dt.float32
NPBF16 = ml_dtypes.bfloat16
NPF8 = ml_dtypes.float8_e4m3
DR = mybir.MatmulPerfMode.DoubleRow

B, N, C = 64, 577, 768
H, D = 12, 64
NCORES = 8
BPC = B // NCORES          # 8 batches per core
CT = C // 128              # 6 chunks of the feature dim
NT = 5                     # token tiles of 128 (last holds 65)
NTAIL = N - 4 * 128        # 65
SCALE = D ** -0.5          # folded into wq on the host
X2PAD = 5 * 128 - N        # 63 rows of row padding after the last batch


def build_module():
    nc = bacc.Bacc("TRN2", target_bir_lowering=False, debug=False)

    xT_d = nc.dram_tensor("xT", [C, BPC, N], F8, kind="ExternalInput")
    x2_d = nc.dram_tensor("x2", [BPC * N, C], F8, kind="ExternalInput")
    wq_d = nc.dram_tensor("wq", [C, C], BF16, kind="ExternalInput")    # [c, o]
    wk2_d = nc.dram_tensor("wk2", [C, C], F8, kind="ExternalInput")    # [o, c]
    wv_d = nc.dram_tensor("wv", [C, C], F8, kind="ExternalInput")      # [c, o]
    wp_d = nc.dram_tensor("wp", [C, C], F8, kind="ExternalInput")      # [c, o]
    xcls_d = nc.dram_tensor("xcls", [C, BPC], BF16, kind="ExternalInput")
    wtqb_d = nc.dram_tensor("wtqb", [C, BPC * H], F8, kind="ExternalInput")
    pbT_d = nc.dram_tensor("pbT", [C, BPC], F32, kind="ExternalInput")
    clsT_d = nc.dram_tensor("clsT", [C, BPC], F32, kind="ExternalOutput")

    AF = mybir.ActivationFunctionType

    with tile.TileContext(nc) as tc:
        with (
            tc.tile_pool(name="sb", bufs=1) as sb,
            tc.tile_pool(name="psA", bufs=2, space="PSUM") as psA,
            tc.tile_pool(name="psW", bufs=1, space="PSUM") as psW,
            tc.tile_pool(name="psS", bufs=1, space="PSUM") as psS,
            tc.tile_pool(name="psR", bufs=1, space="PSUM") as psR,
            tc.tile_pool(name="psZ", bufs=3, space="PSUM") as psZ,
        ):
            # ---- DMAs, in consumption order (one channel, serialized) ----
            wq = sb.tile([128, CT, C], BF16, tag="wq")
            nc.sync.dma_start(
                wq[:], wq_d.ap().rearrange("(a p) o -> p a o", p=128))
            wk2 = sb.tile([128, CT, C], F8, tag="wk2")
            nc.sync.dma_start(
                wk2[:], wk2_d.ap().rearrange("(a p) o -> p a o", p=128))
            xcls = sb.tile([128, CT, BPC], BF16, tag="xcls")
            nc.sync.dma_start(
                xcls[:], xcls_d.ap().rearrange("(a p) b -> p a b", p=128))
            wtqb = sb.tile([128, CT, BPC * H], F8, tag="wtqb")
            nc.sync.dma_start(
                wtqb[:], wtqb_d.ap().rearrange("(a p) o -> p a o", p=128))
            # x in c-major layout, one DMA per batch; rows padded to 640 so
            # DoubleRow k-tile-pair slices have a 64-multiple stride (walrus
            # ISA requirement on Ldweights)
            pbT = sb.tile([128, CT, BPC], F32, tag="pbT")
            nc.sync.dma_start(
                pbT[:], pbT_d.ap().rearrange("(a p) b -> p a b", p=128))
            wv = sb.tile([128, CT, C], F8, tag="wv")
            nc.sync.dma_start(
                wv[:], wv_d.ap().rearrange("(a p) o -> p a o", p=128))
            xTs = []
            for b in range(BPC):
                xt = sb.tile([128, CT, 640], F8, tag=f"xT{b}")
                nc.sync.dma_start(
                    xt[:, :, 0:N],
                    xT_d.ap()[:, b, :].rearrange("(a p) t -> p a t", p=128))
                xTs.append(xt)
            # x in token-major layout, two exact-size DMAs per batch (the
            # 512-row body, then the 65-row tail) so the last batch's Z
            # matmuls mostly run before its tail lands
            x2s = []
            for b in range(BPC):
                x2 = sb.tile([128, NT, C], F8, tag=f"x2{b}")
                nc.sync.dma_start(
                    x2[:, 0:4, :],
                    x2_d.ap()[b * N:b * N + 512, :]
                    .rearrange("(a p) c -> p a c", p=128))
                nc.sync.dma_start(
                    x2[0:NTAIL, 4, :],
                    x2_d.ap()[b * N + 512:b * N + N, :])
                x2s.append(x2)
            # wp is the LAST input: everything up to oT overlaps the input
            # stream, so the only post-stream work is proj -> add -> out DMA
            wp = sb.tile([128, CT, C], F8, tag="wp")
            nc.sync.dma_start(
                wp[:], wp_d.ap().rearrange("(a p) o -> p a o", p=128))

            # ---- small constants ----
            ones8 = sb.tile([128, 2, 64], F8, tag="ones8")
            nc.vector.memset(ones8[:], 1.0)
            negone = sb.tile([128, 1], F32, tag="negone")
            nc.vector.memset(negone[:], -1.0)
            onesf = sb.tile([1, 128], F32, tag="onesf")
            nc.vector.memset(onesf[:], 1.0)
            Qblk = sb.tile([128, CT, 128], F8, tag="Qblk")
            nc.vector.memset(Qblk[:], 0.0)

            # fp8 operand tiles are padded so every DoubleRow k-pair slice
            # has a 64-multiple stride
            Wt = sb.tile([128, CT, 128], F8, tag="Wt")
            pT = sb.tile([128, NT, BPC, 16], F8, tag="pT")
            rden = sb.tile([1, BPC * H], F32, tag="rden")
            rdenB = sb.tile([128, BPC, H], F32, tag="rdenB")
            ZT = sb.tile([128, CT, BPC, 16], F8, tag="ZT")
            oT = sb.tile([128, CT, 64], F8, tag="oT")
            clsT_sb = sb.tile([128, CT, BPC], F32, tag="clsT_sb")

            # ---- qT[o, b]: 36 bf16 matmuls, out free dim 8 ----
            pq = psA.tile([128, CT, BPC], F32, tag="A")
            for oc in range(CT):
                for ck in range(CT):
                    nc.tensor.matmul(
                        pq[:, oc, :],
                        wq[:, ck, 128 * oc:128 * (oc + 1)],
                        xcls[:, ck, :],
                        start=(ck == 0), stop=(ck == CT - 1))

            # ---- Qblk[o, (b h)]: blockdiag scatter of qT (fp8) ----
            QblkV = Qblk[:, :, 0:BPC * H].rearrange(
                "p a (b h) -> p a b h", h=H)
            for oc in range(CT):
                for j in range(2):
                    h = 2 * oc + j
                    nc.vector.tensor_copy(
                        QblkV[64 * j:64 * (j + 1), oc, :, h],
                        pq[64 * j:64 * (j + 1), oc, :])

            # ---- Wt[c, (b h)] = wk2.T @ Qblk + wtqb, cast fp8 ----
            for cj in range(CT):
                pw = psW.tile([128, BPC * H], F32, tag="W")
                for t in range(3):
                    nc.tensor.matmul(
                        pw[:], wk2[:, 2 * t:2 * t + 2, 128 * cj:128 * (cj + 1)],
                        Qblk[:, 2 * t:2 * t + 2, 0:BPC * H],
                        start=(t == 0), stop=(t == 2), perf_mode=DR)
                nc.vector.tensor_add(Wt[:, cj, 0:BPC * H], pw[:], wtqb[:, cj, :])

            # ---- sT[n, (b h)] per batch: 15 DR matmuls over c ----
            ps_s = psS.tile([128, NT, BPC, H], F32, tag="S")
            for b in range(BPC):
                for nt in range(NT):
                    w = 128 if nt < NT - 1 else NTAIL
                    off = 128 * nt
                    for t in range(3):
                        nc.tensor.matmul(
                            ps_s[:w, nt, b, :],
                            xTs[b][:, 2 * t:2 * t + 2, off:off + w],
                            Wt[:, 2 * t:2 * t + 2, H * b:H * (b + 1)],
                            start=(t == 0), stop=(t == 2), perf_mode=DR)

            # ---- pT = exp(sT - 1), fp8 (the -1 cancels in 1/sum and
            #      keeps e below the fp8e4 max) ----
            nc.scalar.activation(
                pT[:, 0:4, :, 0:H], ps_s[:, 0:4, :, :], AF.Exp,
                bias=negone[:], scale=1.0)
            nc.scalar.activation(
                pT[:NTAIL, 4, :, 0:H], ps_s[:NTAIL, 4, :, :], AF.Exp,
                bias=negone[:NTAIL, :], scale=1.0)

            # ---- sums over n via ones-matmuls; rden = 1/sums ----
            pr = psR.tile([128, 192], F32, tag="R")
            for nt in range(NT):
                w = 128 if nt < NT - 1 else NTAIL
                nc.tensor.matmul(
                    pr[0:1, 0:96], ones8[:w, 0, 0:1],
                    pT[:w, nt, :, 0:H],
                    start=(nt == 0), stop=(nt == NT - 1))
            nc.vector.reciprocal(rden[:], pr[0:1, 0:96])

            # ---- rdenB[o, (b h)]: broadcast rden down 128 partitions with
            #      an outer-product matmul ----
            nc.tensor.matmul(
                pr[:, 96:192], onesf[:], rden[:], start=True, stop=True)
            nc.vector.tensor_copy(
                rdenB[:].rearrange("p b h -> p (b h)"), pr[:, 96:192])

            # ---- ZT[c, b-col] per batch: 18 DR matmuls + normalize-and-
            #      cast evacuation (runs as each x2 batch lands) ----
            po = psA.tile([128, CT, BPC], F32, tag="A")
            for b in range(BPC):
                pz = psZ.tile([128, CT, H], F32, tag="Z")
                x2 = x2s[b]
                for ci in range(CT):
                    for t in range(2):
                        nc.tensor.matmul(
                            pz[:, ci, :],
                            x2[:, 2 * t:2 * t + 2, 128 * ci:128 * (ci + 1)],
                            pT[:, 2 * t:2 * t + 2, b, 0:H],
                            start=(t == 0), stop=False, perf_mode=DR)
                    nc.tensor.matmul(
                        pz[:, ci, :],
                        x2[:NTAIL, 4, 128 * ci:128 * (ci + 1)],
                        pT[:NTAIL, 4, b, 0:H],
                        start=False, stop=True)
                nc.vector.tensor_mul(
                    ZT[:, :, b, 0:H], pz[:],
                    rdenB[:, b:b + 1, :].to_broadcast([128, CT, H]))

            # ---- oT per group (4/3/1 batches) so it tracks x2 arrivals.
            # non-DR: DoubleRow + dst partition 64 fails the walrus ISA
            # check (s3d3_mm_valid_dst_partition); cost is per-out-column
            # anyway so plain fp8 matmuls are the same speed here ----
            for js, jn in ((0, 4), (4, 3), (7, 1)):
                for ci in range(CT):
                    for hh in range(2):
                        h = 2 * ci + hh
                        base = 128 * ci + 64 * hh
                        for t in range(CT):
                            nc.tensor.matmul(
                                po[64 * hh:64 * (hh + 1), ci, js:js + jn],
                                wv[:, t, base:base + 64],
                                ZT[:, t, js:js + jn, h],
                                start=(t == 0), stop=(t == CT - 1),
                                tile_position=(0, 64 * hh))
                nc.vector.tensor_copy(
                    oT[:, :, js:js + jn], po[:, :, js:js + jn])

            # ---- clsT[j, b] = wp.T @ oT + pbT: the only work that waits
            #      for wp (the last DMA); one add, one output DMA ----
            pc = psA.tile([128, CT, BPC], F32, tag="A")
            for jc in range(CT):
                for t in range(3):
                    nc.tensor.matmul(
                        pc[:, jc, :],
                        wp[:, 2 * t:2 * t + 2, 128 * jc:128 * (jc + 1)],
                        oT[:, 2 * t:2 * t + 2, 0:BPC],
                        start=(t == 0), stop=(t == 2), perf_mode=DR)
            nc.vector.tensor_add(clsT_sb[:], pc[:], pbT[:])
            nc.sync.dma_start(
                clsT_d.ap().rearrange("(a p) b -> p a b", p=128),
                clsT_sb[:])

    nc.compile()
    return nc


@functools.lru_cache(maxsize=1)
def _module():
    return build_module()


def make_in_maps(x, qkv_w, qkv_b, proj_w, proj_b):
    x = np.asarray(x, dtype=np.float32)
    qkv_w = np.asarray(qkv_w, dtype=np.float32)
    qkv_b = np.asarray(qkv_b, dtype=np.float32)
    proj_w = np.asarray(proj_w, dtype=np.float32)
    proj_b = np.asarray(proj_b, dtype=np.float32)

    wq = np.ascontiguousarray(qkv_w[:C].T * SCALE).astype(NPBF16)   # [c, o]
    wk2 = np.ascontiguousarray(qkv_w[C:2 * C]).astype(NPF8)         # [o, c]
    wv = np.ascontiguousarray(qkv_w[2 * C:].T).astype(NPF8)         # [c, o]
    wp = np.ascontiguousarray(proj_w.T).astype(NPF8)                # [c, o]
    # q-bias folds into Wt: wtqb[c, h] = wk_block_h[:, c] . qb_block_h
    qbs = qkv_b[:C] * SCALE
    wtqb1 = np.stack(
        [qkv_w[C + 64 * h:C + 64 * (h + 1)].T @ qbs[64 * h:64 * (h + 1)]
         for h in range(H)], axis=1)                                # [C, H]
    wtqb = np.tile(wtqb1, (1, BPC)).astype(NPF8)                    # [C, 96]
    # v bias contributes exactly (vb @ proj_w.T) to cls; fold into proj bias
    pb_eff = proj_b + qkv_b[2 * C:] @ proj_w.T

    in_maps = []
    for i in range(NCORES):
        xs = x[i * BPC:(i + 1) * BPC]                               # [8, N, C]
        x2 = xs.reshape(BPC * N, C).astype(NPF8)
        xT = np.ascontiguousarray(xs.transpose(2, 0, 1)).astype(NPF8)
        xcls = np.ascontiguousarray(xs[:, 0, :].T).astype(NPBF16)   # [C, 8]
        pbT = np.tile(pb_eff[:, None], (1, BPC)).astype(np.float32)
        in_maps.append({
            "xT": xT, "x2": x2, "wq": wq, "wk2": wk2, "wv": wv, "wp": wp,
            "xcls": xcls, "wtqb": wtqb, "pbT": pbT,
        })
    return in_maps


def kernel(x, qkv_w, qkv_b, proj_w, proj_b):
    nc = _module()
    in_maps = make_in_maps(x, qkv_w, qkv_b, proj_w, proj_b)
    res = bass_utils.run_bass_kernel_spmd(
        nc, in_maps, core_ids=list(range(NCORES)))
    out = np.array(np.asarray(x), dtype=np.float32, copy=True)
    for i in range(NCORES):
        out[i * BPC:(i + 1) * BPC, 0, :] = res.results[i]["clsT"].T
    return out


# revision 31
# speedup vs baseline: 1.1801x; 1.0292x over previous
"""ClassAttention kernel for 8x TRN2 NeuronCores — fp8 DoubleRow rewrite.

Reference computation (per batch element):
    qkv = x @ qkv_w.T + qkv_b                      # [N, 3C]
    q, k, v = split(qkv)                           # heads H=12, D=64
    s = softmax((q_cls . k) / sqrt(D))             # class-token query only
    cls = (s @ v) @ proj_w.T + proj_b              # [1, C]
    out = concat([cls, x[1:]])                     # rows 1..N pass through

Only the class token row changes, so the device computes just the [B, C]
cls output (shipped transposed as clsT [C, B]); rows 1..N pass through on
the host.  Data-parallel over batch: 8 batches per core, no collectives.

Algebraic structure (inherited from the bf16 baseline):
  - k-projection folds into x-space:  s[b,h,n] = sum_c Wt[c,bh] x[b,n,c]
    with Wt = wk.T @ blockdiag(q) computed once on device; no k vector is
    materialized.  k-bias cancels in softmax; q-bias folds into Wt via a
    host-precomputed wtqb.
  - v-projection commutes with the attention average: the kernel averages
    x (ZT = x.T @ p) and projects through wv once; v-bias folds into the
    proj bias on the host.
  - softmax skips the max-shift (scores are O(1)); the 1/sum scaling is
    applied per (b,h) column during the ZT psum evacuation.

What is new vs the baseline (82.2us -> ~35us modeled):
  - fp8(e4m3) data path: x (both layouts), wv, wp, Wt, p=exp(s), ZT, oT
    are fp8; the score-weight path (wq, wk2, q, Qblk, Wt accumulation)
    stays bf16 because it dominates the error budget.  DoubleRow fp8
    matmuls (2 K-tiles per instruction, 0.5 cycles/row) carry all the
    heavy contractions.
  - every stage computes the TRANSPOSED output with a small moving free
    dim (qT, sT, ZT, oT, clsT), so there are ZERO data transposes and
    psum evacuations are few and wide ([128, .] copies, not [12, .]).
  - 21 large DMAs instead of 67 (HWDGE issue cost ~630ns each gated the
    baseline); x2 is read as [128, 5, 768] per batch from a 63-row-padded
    flat buffer so each batch is one descriptor-dense transfer.

Per-core dataflow (b = 0..8 batches, c in 6 chunks of 128):
  qT[o, b]        36 bf16 matmuls      (needs xcls, wq)
  Qblk[o, (b h)]  12 blockdiag copies  (DVE, psum -> bf16)
  Wt[c, (b h)]    36 bf16 matmuls + 6 adds (+wtqb, cast fp8)
  sT[n, (b h)]    120 DR matmuls       (needs all xT)
  pT = exp(sT-1)  2 Act ops, fp8       (bias cancels in the 1/sum)
  sums[1, (b h)]  3 ones-matmuls; rden = 1/sums (f32)
  rdenB[o, (b h)] 2 outer-product matmuls + copy
  ZT[c, (g j h)]  144 DR matmuls       (needs x2_b), x rden -> fp8
  oT[o', b]       72 DR matmuls        (diag blocks direct, needs wv)
  clsT[j, b]      36 DR matmuls + pbT add -> f32, DMA out per group
"""

import functools

import numpy as np
import ml_dtypes

import concourse.bass as bass
import concourse.tile as tile
from concourse import bacc, mybir
from concourse import bass_utils

BF16 = mybir.dt.bfloat16
F8 = mybir.dt.float8e4
F32 = mybir.dt.float32
NPBF16 = ml_dtypes.bfloat16
NPF8 = ml_dtypes.float8_e4m3
DR = mybir.MatmulPerfMode.DoubleRow

B, N, C = 64, 577, 768
H, D = 12, 64
NCORES = 8
BPC = B // NCORES          # 8 batches per core
CT = C // 128              # 6 chunks of the feature dim
NT = 5                     # token tiles of 128 (last holds 65)
NTAIL = N - 4 * 128        # 65
SCALE = D ** -0.5          # folded into wq on the host
X2PAD = 5 * 128 - N        # 63 rows of row padding after the last batch


def build_module():
    nc = bacc.Bacc("TRN2", target_bir_lowering=False, debug=False)

    xT_d = nc.dram_tensor("xT", [C, BPC, N], F8, kind="ExternalInput")
    x2_d = nc.dram_tensor("x2", [BPC * N, C], F8, kind="ExternalInput")
    wq_d = nc.dram_tensor("wq", [C, C], BF16, kind="ExternalInput")    # [c, o]
    wk2_d = nc.dram_tensor("wk2", [C, C], F8, kind="ExternalInput")    # [o, c]
    wv_d = nc.dram_tensor("wv", [C, C], F8, kind="ExternalInput")      # [c, o]
    wp_d = nc.dram_tensor("wp", [C, C], F8, kind="ExternalInput")      # [c, o]
    xcls_d = nc.dram_tensor("xcls", [128, CT, BPC], BF16, kind="ExternalInput")
    wtqb_d = nc.dram_tensor("wtqb", [128, CT, BPC * H], F8, kind="ExternalInput")
    pbT_d = nc.dram_tensor("pbT", [128, CT, BPC], F32, kind="ExternalInput")
    clsT_d = nc.dram_tensor("clsT", [128, CT, BPC], F32, kind="ExternalOutput")

    AF = mybir.ActivationFunctionType

    with tile.TileContext(nc) as tc:
        with (
            tc.tile_pool(name="sb", bufs=1) as sb,
            tc.tile_pool(name="psA", bufs=2, space="PSUM") as psA,
            tc.tile_pool(name="psW", bufs=1, space="PSUM") as psW,
            tc.tile_pool(name="psS", bufs=1, space="PSUM") as psS,
            tc.tile_pool(name="psR", bufs=1, space="PSUM") as psR,
            tc.tile_pool(name="psZ", bufs=3, space="PSUM") as psZ,
        ):
            # ---- DMAs, in consumption order (one channel, serialized) ----
            wq = sb.tile([128, CT, C], BF16, tag="wq")
            nc.sync.dma_start(
                wq[:], wq_d.ap().rearrange("(a p) o -> p a o", p=128))
            wk2 = sb.tile([128, CT, C], F8, tag="wk2")
            nc.sync.dma_start(
                wk2[:], wk2_d.ap().rearrange("(a p) o -> p a o", p=128))
            xcls = sb.tile([128, CT, BPC], BF16, tag="xcls")
            nc.sync.dma_start(xcls[:], xcls_d.ap())
            wtqb = sb.tile([128, CT, BPC * H], F8, tag="wtqb")
            nc.sync.dma_start(wtqb[:], wtqb_d.ap())
            # x in c-major layout, one DMA per batch; rows padded to 640 so
            # DoubleRow k-tile-pair slices have a 64-multiple stride (walrus
            # ISA requirement on Ldweights)
            wv = sb.tile([128, CT, C], F8, tag="wv")
            nc.sync.dma_start(
                wv[:], wv_d.ap().rearrange("(a p) o -> p a o", p=128))
            xTs = []
            for b in range(BPC):
                xt = sb.tile([128, CT, 640], F8, tag=f"xT{b}")
                nc.sync.dma_start(
                    xt[:, :, 0:N],
                    xT_d.ap()[:, b, :].rearrange("(a p) t -> p a t", p=128))
                xTs.append(xt)
            # x in token-major layout, two exact-size DMAs per batch (the
            # 512-row body, then the 65-row tail) so the last batch's Z
            # matmuls mostly run before its tail lands
            x2s = []
            for b in range(BPC):
                x2 = sb.tile([128, NT, C], F8, tag=f"x2{b}")
                nc.sync.dma_start(
                    x2[:, 0:4, :],
                    x2_d.ap()[b * N:b * N + 512, :]
                    .rearrange("(a p) c -> p a c", p=128))
                nc.sync.dma_start(
                    x2[0:NTAIL, 4, :],
                    x2_d.ap()[b * N + 512:b * N + N, :])
                x2s.append(x2)
            # wp is the LAST input: everything up to oT overlaps the input
            # stream, so the only post-stream work is proj -> add -> out DMA
            wp = sb.tile([128, CT, C], F8, tag="wp")
            nc.sync.dma_start(
                wp[:], wp_d.ap().rearrange("(a p) o -> p a o", p=128))
            # pbT is the very last input: the only work behind it is the
            # final bias add
            pbT = sb.tile([128, CT, BPC], F32, tag="pbT")
            nc.sync.dma_start(pbT[:], pbT_d.ap())

            # ---- small constants ----
            ones8 = sb.tile([128, 2, 64], F8, tag="ones8")
            nc.vector.memset(ones8[:], 1.0)
            negone = sb.tile([128, 1], F32, tag="negone")
            nc.vector.memset(negone[:], -1.0)
            onesf = sb.tile([1, 128], F32, tag="onesf")
            nc.vector.memset(onesf[:], 1.0)
            Qblk = sb.tile([128, CT, 128], F8, tag="Qblk")
            nc.vector.memset(Qblk[:], 0.0)

            # fp8 operand tiles are padded so every DoubleRow k-pair slice
            # has a 64-multiple stride
            Wt = sb.tile([128, CT, 128], F8, tag="Wt")
            pT = sb.tile([128, NT, BPC, 16], F8, tag="pT")
            rden = sb.tile([1, BPC * H], F32, tag="rden")
            rdenB = sb.tile([128, BPC, H], F32, tag="rdenB")
            ZT = sb.tile([128, CT, BPC, 16], F8, tag="ZT")
            oT = sb.tile([128, CT, 64], F8, tag="oT")
            clsT_sb = sb.tile([128, CT, BPC], F32, tag="clsT_sb")

            # ---- qT[o, b]: 36 bf16 matmuls, out free dim 8 ----
            pq = psA.tile([128, CT, BPC], F32, tag="A")
            for oc in range(CT):
                for ck in range(CT):
                    nc.tensor.matmul(
                        pq[:, oc, :],
                        wq[:, ck, 128 * oc:128 * (oc + 1)],
                        xcls[:, ck, :],
                        start=(ck == 0), stop=(ck == CT - 1))

            # ---- Qblk[o, (b h)]: blockdiag scatter of qT (fp8) ----
            QblkV = Qblk[:, :, 0:BPC * H].rearrange(
                "p a (b h) -> p a b h", h=H)
            for oc in range(CT):
                for j in range(2):
                    h = 2 * oc + j
                    nc.vector.tensor_copy(
                        QblkV[64 * j:64 * (j + 1), oc, :, h],
                        pq[64 * j:64 * (j + 1), oc, :])

            # ---- Wt[c, (b h)] = wk2.T @ Qblk + wtqb, cast fp8 ----
            for cj in range(CT):
                pw = psW.tile([128, BPC * H], F32, tag="W")
                for t in range(3):
                    nc.tensor.matmul(
                        pw[:], wk2[:, 2 * t:2 * t + 2, 128 * cj:128 * (cj + 1)],
                        Qblk[:, 2 * t:2 * t + 2, 0:BPC * H],
                        start=(t == 0), stop=(t == 2), perf_mode=DR)
                nc.vector.tensor_add(Wt[:, cj, 0:BPC * H], pw[:], wtqb[:, cj, :])

            # ---- sT[n, (b h)] per batch: 15 DR matmuls over c ----
            ps_s = psS.tile([128, NT, BPC, H], F32, tag="S")
            for b in range(BPC):
                for nt in range(NT):
                    w = 128 if nt < NT - 1 else NTAIL
                    off = 128 * nt
                    for t in range(3):
                        nc.tensor.matmul(
                            ps_s[:w, nt, b, :],
                            xTs[b][:, 2 * t:2 * t + 2, off:off + w],
                            Wt[:, 2 * t:2 * t + 2, H * b:H * (b + 1)],
                            start=(t == 0), stop=(t == 2), perf_mode=DR)

            # ---- pT = exp(sT - 1), fp8 (the -1 cancels in 1/sum and
            #      keeps e below the fp8e4 max) ----
            nc.scalar.activation(
                pT[:, 0:4, :, 0:H], ps_s[:, 0:4, :, :], AF.Exp,
                bias=negone[:], scale=1.0)
            nc.scalar.activation(
                pT[:NTAIL, 4, :, 0:H], ps_s[:NTAIL, 4, :, :], AF.Exp,
                bias=negone[:NTAIL, :], scale=1.0)

            # ---- sums over n via ones-matmuls; rden = 1/sums ----
            pr = psR.tile([128, 192], F32, tag="R")
            for nt in range(NT):
                w = 128 if nt < NT - 1 else NTAIL
                nc.tensor.matmul(
                    pr[0:1, 0:96], ones8[:w, 0, 0:1],
                    pT[:w, nt, :, 0:H],
                    start=(nt == 0), stop=(nt == NT - 1))
            nc.vector.reciprocal(rden[:], pr[0:1, 0:96])

            # ---- rdenB[o, (b h)]: broadcast rden down 128 partitions with
            #      an outer-product matmul ----
            nc.tensor.matmul(
                pr[:, 96:192], onesf[:], rden[:], start=True, stop=True)
            nc.vector.tensor_copy(
                rdenB[:].rearrange("p b h -> p (b h)"), pr[:, 96:192])

            # ---- ZT[c, b-col] per batch: 18 DR matmuls + normalize-and-
            #      cast evacuation (runs as each x2 batch lands) ----
            po = psA.tile([128, CT, BPC], F32, tag="A")
            for b in range(BPC):
                pz = psZ.tile([128, CT, H], F32, tag="Z")
                x2 = x2s[b]
                for ci in range(CT):
                    for t in range(2):
                        nc.tensor.matmul(
                            pz[:, ci, :],
                            x2[:, 2 * t:2 * t + 2, 128 * ci:128 * (ci + 1)],
                            pT[:, 2 * t:2 * t + 2, b, 0:H],
                            start=(t == 0), stop=False, perf_mode=DR)
                    nc.tensor.matmul(
                        pz[:, ci, :],
                        x2[:NTAIL, 4, 128 * ci:128 * (ci + 1)],
                        pT[:NTAIL, 4, b, 0:H],
                        start=False, stop=True)
                nc.vector.tensor_mul(
                    ZT[:, :, b, 0:H], pz[:],
                    rdenB[:, b:b + 1, :].to_broadcast([128, CT, H]))

            # ---- oT per group (4/3/1 batches) so it tracks x2 arrivals.
            # non-DR: DoubleRow + dst partition 64 fails the walrus ISA
            # check (s3d3_mm_valid_dst_partition); cost is per-out-column
            # anyway so plain fp8 matmuls are the same speed here ----
            for js, jn in ((0, 4), (4, 3), (7, 1)):
                for ci in range(CT):
                    for hh in range(2):
                        h = 2 * ci + hh
                        base = 128 * ci + 64 * hh
                        for t in range(CT):
                            nc.tensor.matmul(
                                po[64 * hh:64 * (hh + 1), ci, js:js + jn],
                                wv[:, t, base:base + 64],
                                ZT[:, t, js:js + jn, h],
                                start=(t == 0), stop=(t == CT - 1),
                                tile_position=(0, 64 * hh))
                nc.vector.tensor_copy(
                    oT[:, :, js:js + jn], po[:, :, js:js + jn])

            # ---- clsT[j, b] = wp.T @ oT + pbT: the only work that waits
            #      for wp (the last DMA); one add, one output DMA ----
            pc = psA.tile([128, CT, BPC], F32, tag="A")
            for jc in range(CT):
                for t in range(3):
                    nc.tensor.matmul(
                        pc[:, jc, :],
                        wp[:, 2 * t:2 * t + 2, 128 * jc:128 * (jc + 1)],
                        oT[:, 2 * t:2 * t + 2, 0:BPC],
                        start=(t == 0), stop=(t == 2), perf_mode=DR)
            nc.vector.tensor_add(clsT_sb[:], pc[:], pbT[:])
            nc.sync.dma_start(clsT_d.ap(), clsT_sb[:])

    nc.compile()
    return nc


@functools.lru_cache(maxsize=1)
def _module():
    return build_module()


def make_in_maps(x, qkv_w, qkv_b, proj_w, proj_b):
    x = np.asarray(x, dtype=np.float32)
    qkv_w = np.asarray(qkv_w, dtype=np.float32)
    qkv_b = np.asarray(qkv_b, dtype=np.float32)
    proj_w = np.asarray(proj_w, dtype=np.float32)
    proj_b = np.asarray(proj_b, dtype=np.float32)

    wq = np.ascontiguousarray(qkv_w[:C].T * SCALE).astype(NPBF16)   # [c, o]
    wk2 = np.ascontiguousarray(qkv_w[C:2 * C]).astype(NPF8)         # [o, c]
    wv = np.ascontiguousarray(qkv_w[2 * C:].T).astype(NPF8)         # [c, o]
    wp = np.ascontiguousarray(proj_w.T).astype(NPF8)                # [c, o]
    # q-bias folds into Wt: wtqb[c, h] = wk_block_h[:, c] . qb_block_h
    qbs = qkv_b[:C] * SCALE
    wtqb1 = np.stack(
        [qkv_w[C + 64 * h:C + 64 * (h + 1)].T @ qbs[64 * h:64 * (h + 1)]
         for h in range(H)], axis=1)                                # [C, H]
    wtqb = np.ascontiguousarray(
        np.tile(wtqb1, (1, BPC)).reshape(CT, 128, BPC * H)
        .transpose(1, 0, 2)).astype(NPF8)                           # [p, a, 96]
    # v bias contributes exactly (vb @ proj_w.T) to cls; fold into proj bias
    pb_eff = proj_b + qkv_b[2 * C:] @ proj_w.T

    in_maps = []
    for i in range(NCORES):
        xs = x[i * BPC:(i + 1) * BPC]                               # [8, N, C]
        x2 = xs.reshape(BPC * N, C).astype(NPF8)
        xT = np.ascontiguousarray(xs.transpose(2, 0, 1)).astype(NPF8)
        xcls = np.ascontiguousarray(
            xs[:, 0, :].T.reshape(CT, 128, BPC).transpose(1, 0, 2)
        ).astype(NPBF16)                                            # [p, a, b]
        pbT = np.ascontiguousarray(
            np.tile(pb_eff[:, None], (1, BPC)).reshape(CT, 128, BPC)
            .transpose(1, 0, 2)).astype(np.float32)                 # [p, a, b]
        in_maps.append({
            "xT": xT, "x2": x2, "wq": wq, "wk2": wk2, "wv": wv, "wp": wp,
            "xcls": xcls, "wtqb": wtqb, "pbT": pbT,
        })
    return in_maps


def kernel(x, qkv_w, qkv_b, proj_w, proj_b):
    nc = _module()
    in_maps = make_in_maps(x, qkv_w, qkv_b, proj_w, proj_b)
    res = bass_utils.run_bass_kernel_spmd(
        nc, in_maps, core_ids=list(range(NCORES)))
    out = np.array(np.asarray(x), dtype=np.float32, copy=True)
    for i in range(NCORES):
        clsT = res.results[i]["clsT"]                               # [p, a, b]
        out[i * BPC:(i + 1) * BPC, 0, :] = (
            clsT.transpose(2, 1, 0).reshape(BPC, C))
    return out


# revision 32
# speedup vs baseline: 1.2393x; 1.0502x over previous
"""ClassAttention kernel for 8x TRN2 NeuronCores — fp8 DoubleRow rewrite.

Reference computation (per batch element):
    qkv = x @ qkv_w.T + qkv_b                      # [N, 3C]
    q, k, v = split(qkv)                           # heads H=12, D=64
    s = softmax((q_cls . k) / sqrt(D))             # class-token query only
    cls = (s @ v) @ proj_w.T + proj_b              # [1, C]
    out = concat([cls, x[1:]])                     # rows 1..N pass through

Only the class token row changes, so the device computes just the [B, C]
cls output (shipped transposed as clsT [C, B]); rows 1..N pass through on
the host.  Data-parallel over batch: 8 batches per core, no collectives.

Algebraic structure (inherited from the bf16 baseline):
  - k-projection folds into x-space:  s[b,h,n] = sum_c Wt[c,bh] x[b,n,c]
    with Wt = wk.T @ blockdiag(q) computed once on device; no k vector is
    materialized.  k-bias cancels in softmax; q-bias folds into Wt via a
    host-precomputed wtqb.
  - v-projection commutes with the attention average: the kernel averages
    x (ZT = x.T @ p) and projects through wv once; v-bias folds into the
    proj bias on the host.
  - softmax skips the max-shift (scores are O(1)); the 1/sum scaling is
    applied per (b,h) column during the ZT psum evacuation.

What is new vs the baseline (82.2us -> ~35us modeled):
  - fp8(e4m3) data path: x (both layouts), wv, wp, Wt, p=exp(s), ZT, oT
    are fp8; the score-weight path (wq, wk2, q, Qblk, Wt accumulation)
    stays bf16 because it dominates the error budget.  DoubleRow fp8
    matmuls (2 K-tiles per instruction, 0.5 cycles/row) carry all the
    heavy contractions.
  - every stage computes the TRANSPOSED output with a small moving free
    dim (qT, sT, ZT, oT, clsT), so there are ZERO data transposes and
    psum evacuations are few and wide ([128, .] copies, not [12, .]).
  - 21 large DMAs instead of 67 (HWDGE issue cost ~630ns each gated the
    baseline); x2 is read as [128, 5, 768] per batch from a 63-row-padded
    flat buffer so each batch is one descriptor-dense transfer.

Per-core dataflow (b = 0..8 batches, c in 6 chunks of 128):
  qT[o, b]        36 bf16 matmuls      (needs xcls, wq)
  Qblk[o, (b h)]  12 blockdiag copies  (DVE, psum -> bf16)
  Wt[c, (b h)]    36 bf16 matmuls + 6 adds (+wtqb, cast fp8)
  sT[n, (b h)]    120 DR matmuls       (needs all xT)
  pT = exp(sT-1)  2 Act ops, fp8       (bias cancels in the 1/sum)
  sums[1, (b h)]  3 ones-matmuls; rden = 1/sums (f32)
  rdenB[o, (b h)] 2 outer-product matmuls + copy
  ZT[c, (g j h)]  144 DR matmuls       (needs x2_b), x rden -> fp8
  oT[o', b]       72 DR matmuls        (diag blocks direct, needs wv)
  clsT[j, b]      36 DR matmuls + pbT add -> f32, DMA out per group
"""

import functools

import numpy as np
import ml_dtypes

import concourse.bass as bass
import concourse.tile as tile
from concourse import bacc, mybir
from concourse import bass_utils

BF16 = mybir.dt.bfloat16
F8 = mybir.dt.float8e4
F32 = mybir.dt.float32
NPBF16 = ml_dtypes.bfloat16
NPF8 = ml_dtypes.float8_e4m3
DR = mybir.MatmulPerfMode.DoubleRow

B, N, C = 64, 577, 768
H, D = 12, 64
NCORES = 8
BPC = B // NCORES          # 8 batches per core
CT = C // 128              # 6 chunks of the feature dim
NT = 5                     # token tiles of 128 (last holds 65)
NTAIL = N - 4 * 128        # 65
SCALE = D ** -0.5          # folded into wq on the host
X2PAD = 5 * 128 - N        # 63 rows of row padding after the last batch


def build_module():
    nc = bacc.Bacc("TRN2", target_bir_lowering=False, debug=False)

    xT_d = nc.dram_tensor("xT", [C, BPC, N], F8, kind="ExternalInput")
    x2_d = nc.dram_tensor("x2", [BPC * N, C], F8, kind="ExternalInput")
    wq_d = nc.dram_tensor("wq", [C, C], F8, kind="ExternalInput")      # [c, o]
    wk2_d = nc.dram_tensor("wk2", [C, C], F8, kind="ExternalInput")    # [o, c]
    wv_d = nc.dram_tensor("wv", [C, C], F8, kind="ExternalInput")      # [c, o]
    wp_d = nc.dram_tensor("wp", [C, C], F8, kind="ExternalInput")      # [c, o]
    xcls_d = nc.dram_tensor("xcls", [128, CT, BPC], F8, kind="ExternalInput")
    wtqb_d = nc.dram_tensor("wtqb", [128, CT, BPC * H], F8, kind="ExternalInput")
    pbT_d = nc.dram_tensor("pbT", [128, CT, BPC], F32, kind="ExternalInput")
    clsT_d = nc.dram_tensor("clsT", [128, CT, BPC], F32, kind="ExternalOutput")

    AF = mybir.ActivationFunctionType

    with tile.TileContext(nc) as tc:
        with (
            tc.tile_pool(name="sb", bufs=1) as sb,
            tc.tile_pool(name="psA", bufs=2, space="PSUM") as psA,
            tc.tile_pool(name="psW", bufs=1, space="PSUM") as psW,
            tc.tile_pool(name="psS", bufs=1, space="PSUM") as psS,
            tc.tile_pool(name="psR", bufs=1, space="PSUM") as psR,
            tc.tile_pool(name="psZ", bufs=3, space="PSUM") as psZ,
        ):
            # ---- DMAs, in consumption order (one channel, serialized) ----
            wq = sb.tile([128, CT, C], F8, tag="wq")
            nc.sync.dma_start(
                wq[:], wq_d.ap().rearrange("(a p) o -> p a o", p=128))
            wk2 = sb.tile([128, CT, C], F8, tag="wk2")
            nc.sync.dma_start(
                wk2[:], wk2_d.ap().rearrange("(a p) o -> p a o", p=128))
            xcls = sb.tile([128, CT, BPC], F8, tag="xcls")
            nc.sync.dma_start(xcls[:], xcls_d.ap())
            wtqb = sb.tile([128, CT, BPC * H], F8, tag="wtqb")
            nc.sync.dma_start(wtqb[:], wtqb_d.ap())
            # x in c-major layout, one DMA per batch; rows padded to 640 so
            # DoubleRow k-tile-pair slices have a 64-multiple stride (walrus
            # ISA requirement on Ldweights)
            wv = sb.tile([128, CT, C], F8, tag="wv")
            nc.sync.dma_start(
                wv[:], wv_d.ap().rearrange("(a p) o -> p a o", p=128))
            xTs = []
            for b in range(BPC):
                xt = sb.tile([128, CT, 640], F8, tag=f"xT{b}")
                nc.sync.dma_start(
                    xt[:, :, 0:N],
                    xT_d.ap()[:, b, :].rearrange("(a p) t -> p a t", p=128))
                xTs.append(xt)
            # x in token-major layout, two exact-size DMAs per batch (the
            # 512-row body, then the 65-row tail) so the last batch's Z
            # matmuls mostly run before its tail lands
            x2s = []
            for b in range(BPC):
                x2 = sb.tile([128, NT, C], F8, tag=f"x2{b}")
                nc.sync.dma_start(
                    x2[:, 0:4, :],
                    x2_d.ap()[b * N:b * N + 512, :]
                    .rearrange("(a p) c -> p a c", p=128))
                nc.sync.dma_start(
                    x2[0:NTAIL, 4, :],
                    x2_d.ap()[b * N + 512:b * N + N, :])
                x2s.append(x2)
            # wp is the LAST input: everything up to oT overlaps the input
            # stream, so the only post-stream work is proj -> add -> out DMA
            wp = sb.tile([128, CT, C], F8, tag="wp")
            nc.sync.dma_start(
                wp[:], wp_d.ap().rearrange("(a p) o -> p a o", p=128))
            # pbT is the very last input: the only work behind it is the
            # final bias add
            pbT = sb.tile([128, CT, BPC], F32, tag="pbT")
            nc.sync.dma_start(pbT[:], pbT_d.ap())

            # ---- small constants ----
            ones8 = sb.tile([128, 2, 64], F8, tag="ones8")
            nc.vector.memset(ones8[:], 1.0)
            negone = sb.tile([128, 1], F32, tag="negone")
            nc.vector.memset(negone[:], -1.0)
            onesf = sb.tile([1, 128], F32, tag="onesf")
            nc.vector.memset(onesf[:], 1.0)
            Qblk = sb.tile([128, CT, 128], F8, tag="Qblk")
            nc.vector.memset(Qblk[:], 0.0)

            # fp8 operand tiles are padded so every DoubleRow k-pair slice
            # has a 64-multiple stride
            Wt = sb.tile([128, CT, 128], F8, tag="Wt")
            pT = sb.tile([128, NT, BPC, 16], F8, tag="pT")
            rden = sb.tile([1, BPC * H], F32, tag="rden")
            rdenB = sb.tile([128, BPC, H], F32, tag="rdenB")
            ZT = sb.tile([128, CT, BPC, 16], F8, tag="ZT")
            oT = sb.tile([128, CT, 64], F8, tag="oT")
            clsT_sb = sb.tile([128, CT, BPC], F32, tag="clsT_sb")

            # ---- qT[o, b]: 36 bf16 matmuls, out free dim 8 ----
            pq = psA.tile([128, CT, BPC], F32, tag="A")
            for oc in range(CT):
                for ck in range(CT):
                    nc.tensor.matmul(
                        pq[:, oc, :],
                        wq[:, ck, 128 * oc:128 * (oc + 1)],
                        xcls[:, ck, :],
                        start=(ck == 0), stop=(ck == CT - 1))

            # ---- Qblk[o, (b h)]: blockdiag scatter of qT (fp8) ----
            QblkV = Qblk[:, :, 0:BPC * H].rearrange(
                "p a (b h) -> p a b h", h=H)
            for oc in range(CT):
                for j in range(2):
                    h = 2 * oc + j
                    nc.vector.tensor_copy(
                        QblkV[64 * j:64 * (j + 1), oc, :, h],
                        pq[64 * j:64 * (j + 1), oc, :])

            # ---- Wt[c, (b h)] = wk2.T @ Qblk + wtqb, cast fp8 ----
            for cj in range(CT):
                pw = psW.tile([128, BPC * H], F32, tag="W")
                for t in range(3):
                    nc.tensor.matmul(
                        pw[:], wk2[:, 2 * t:2 * t + 2, 128 * cj:128 * (cj + 1)],
                        Qblk[:, 2 * t:2 * t + 2, 0:BPC * H],
                        start=(t == 0), stop=(t == 2), perf_mode=DR)
                nc.vector.tensor_add(Wt[:, cj, 0:BPC * H], pw[:], wtqb[:, cj, :])

            # ---- sT[n, (b h)] per batch: 15 DR matmuls over c ----
            ps_s = psS.tile([128, NT, BPC, H], F32, tag="S")
            for b in range(BPC):
                for nt in range(NT):
                    w = 128 if nt < NT - 1 else NTAIL
                    off = 128 * nt
                    for t in range(3):
                        nc.tensor.matmul(
                            ps_s[:w, nt, b, :],
                            xTs[b][:, 2 * t:2 * t + 2, off:off + w],
                            Wt[:, 2 * t:2 * t + 2, H * b:H * (b + 1)],
                            start=(t == 0), stop=(t == 2), perf_mode=DR)

            # ---- pT = exp(sT - 1), fp8 (the -1 cancels in 1/sum and
            #      keeps e below the fp8e4 max) ----
            nc.scalar.activation(
                pT[:, 0:4, :, 0:H], ps_s[:, 0:4, :, :], AF.Exp,
                bias=negone[:], scale=1.0)
            nc.scalar.activation(
                pT[:NTAIL, 4, :, 0:H], ps_s[:NTAIL, 4, :, :], AF.Exp,
                bias=negone[:NTAIL, :], scale=1.0)

            # ---- sums over n via ones-matmuls; rden = 1/sums ----
            pr = psR.tile([128, 192], F32, tag="R")
            for nt in range(NT):
                w = 128 if nt < NT - 1 else NTAIL
                nc.tensor.matmul(
                    pr[0:1, 0:96], ones8[:w, 0, 0:1],
                    pT[:w, nt, :, 0:H],
                    start=(nt == 0), stop=(nt == NT - 1))
            nc.vector.reciprocal(rden[:], pr[0:1, 0:96])

            # ---- rdenB[o, (b h)]: broadcast rden down 128 partitions with
            #      an outer-product matmul ----
            nc.tensor.matmul(
                pr[:, 96:192], onesf[:], rden[:], start=True, stop=True)
            nc.vector.tensor_copy(
                rdenB[:].rearrange("p b h -> p (b h)"), pr[:, 96:192])

            # ---- ZT[c, b-col] per batch: 18 DR matmuls + normalize-and-
            #      cast evacuation (runs as each x2 batch lands) ----
            po = psA.tile([128, CT, BPC], F32, tag="A")
            for b in range(BPC):
                pz = psZ.tile([128, CT, H], F32, tag="Z")
                x2 = x2s[b]
                for ci in range(CT):
                    for t in range(2):
                        nc.tensor.matmul(
                            pz[:, ci, :],
                            x2[:, 2 * t:2 * t + 2, 128 * ci:128 * (ci + 1)],
                            pT[:, 2 * t:2 * t + 2, b, 0:H],
                            start=(t == 0), stop=False, perf_mode=DR)
                    nc.tensor.matmul(
                        pz[:, ci, :],
                        x2[:NTAIL, 4, 128 * ci:128 * (ci + 1)],
                        pT[:NTAIL, 4, b, 0:H],
                        start=False, stop=True)
                nc.vector.tensor_mul(
                    ZT[:, :, b, 0:H], pz[:],
                    rdenB[:, b:b + 1, :].to_broadcast([128, CT, H]))

            # ---- oT per group (4/3/1 batches) so it tracks x2 arrivals.
            # non-DR: DoubleRow + dst partition 64 fails the walrus ISA
            # check (s3d3_mm_valid_dst_partition); cost is per-out-column
            # anyway so plain fp8 matmuls are the same speed here ----
            for js, jn in ((0, 4), (4, 3), (7, 1)):
                for ci in range(CT):
                    for hh in range(2):
                        h = 2 * ci + hh
                        base = 128 * ci + 64 * hh
                        for t in range(CT):
                            nc.tensor.matmul(
                                po[64 * hh:64 * (hh + 1), ci, js:js + jn],
                                wv[:, t, base:base + 64],
                                ZT[:, t, js:js + jn, h],
                                start=(t == 0), stop=(t == CT - 1),
                                tile_position=(0, 64 * hh))
                nc.vector.tensor_copy(
                    oT[:, :, js:js + jn], po[:, :, js:js + jn])

            # ---- clsT[j, b] = wp.T @ oT + pbT: the only work that waits
            #      for wp (the last DMA); one add, one output DMA ----
            pc = psA.tile([128, CT, BPC], F32, tag="A")
            for jc in range(CT):
                for t in range(3):
                    nc.tensor.matmul(
                        pc[:, jc, :],
                        wp[:, 2 * t:2 * t + 2, 128 * jc:128 * (jc + 1)],
                        oT[:, 2 * t:2 * t + 2, 0:BPC],
                        start=(t == 0), stop=(t == 2), perf_mode=DR)
            nc.vector.tensor_add(clsT_sb[:], pc[:], pbT[:])
            nc.sync.dma_start(clsT_d.ap(), clsT_sb[:])

    nc.compile()
    return nc


@functools.lru_cache(maxsize=1)
def _module():
    return build_module()


def make_in_maps(x, qkv_w, qkv_b, proj_w, proj_b):
    x = np.asarray(x, dtype=np.float32)
    qkv_w = np.asarray(qkv_w, dtype=np.float32)
    qkv_b = np.asarray(qkv_b, dtype=np.float32)
    proj_w = np.asarray(proj_w, dtype=np.float32)
    proj_b = np.asarray(proj_b, dtype=np.float32)

    wq = np.ascontiguousarray(qkv_w[:C].T * SCALE).astype(NPF8)     # [c, o]
    wk2 = np.ascontiguousarray(qkv_w[C:2 * C]).astype(NPF8)         # [o, c]
    wv = np.ascontiguousarray(qkv_w[2 * C:].T).astype(NPF8)         # [c, o]
    wp = np.ascontiguousarray(proj_w.T).astype(NPF8)                # [c, o]
    # q-bias folds into Wt: wtqb[c, h] = wk_block_h[:, c] . qb_block_h
    qbs = qkv_b[:C] * SCALE
    wtqb1 = np.stack(
        [qkv_w[C + 64 * h:C + 64 * (h + 1)].T @ qbs[64 * h:64 * (h + 1)]
         for h in range(H)], axis=1)                                # [C, H]
    wtqb = np.ascontiguousarray(
        np.tile(wtqb1, (1, BPC)).reshape(CT, 128, BPC * H)
        .transpose(1, 0, 2)).astype(NPF8)                           # [p, a, 96]
    # v bias contributes exactly (vb @ proj_w.T) to cls; fold into proj bias
    pb_eff = proj_b + qkv_b[2 * C:] @ proj_w.T

    in_maps = []
    for i in range(NCORES):
        xs = x[i * BPC:(i + 1) * BPC]                               # [8, N, C]
        x2 = xs.reshape(BPC * N, C).astype(NPF8)
        xT = np.ascontiguousarray(xs.transpose(2, 0, 1)).astype(NPF8)
        xcls = np.ascontiguousarray(
            xs[:, 0, :].T.reshape(CT, 128, BPC).transpose(1, 0, 2)
        ).astype(NPF8)                                              # [p, a, b]
        pbT = np.ascontiguousarray(
            np.tile(pb_eff[:, None], (1, BPC)).reshape(CT, 128, BPC)
            .transpose(1, 0, 2)).astype(np.float32)                 # [p, a, b]
        in_maps.append({
            "xT": xT, "x2": x2, "wq": wq, "wk2": wk2, "wv": wv, "wp": wp,
            "xcls": xcls, "wtqb": wtqb, "pbT": pbT,
        })
    return in_maps


def kernel(x, qkv_w, qkv_b, proj_w, proj_b):
    nc = _module()
    in_maps = make_in_maps(x, qkv_w, qkv_b, proj_w, proj_b)
    res = bass_utils.run_bass_kernel_spmd(
        nc, in_maps, core_ids=list(range(NCORES)))
    out = np.array(np.asarray(x), dtype=np.float32, copy=True)
    for i in range(NCORES):
        clsT = res.results[i]["clsT"]                               # [p, a, b]
        out[i * BPC:(i + 1) * BPC, 0, :] = (
            clsT.transpose(2, 1, 0).reshape(BPC, C))
    return out
